# revision 28
# baseline (speedup 1.0000x reference)
"""MultiHeadAttention TRN2 Bass kernel (8 NeuronCores), fp8 DoubleRow edition.

Sharding: core c = (batch b = c//2, query-half = c%2). Each core computes
K/V for its full batch (2048 keys) and attention + output projection + LN
for its 1024 query rows. No collectives; host gathers per-core outputs.

All heavy matmuls run in fp8e4m3 with MatmulPerfMode.DoubleRow (0.5 PE
cycles per output row, 256-wide contraction per instruction):
  - Q/K projections:  out[4head*32dk half, q|m] over D=1024 (4 DR chunks)
  - V projection:     Vaug[m, 4head*65] (col 0 of each head = ones so the
                      softmax denominator lands at partition 0 of OT)
  - scores:           S[m, q] per head, dk=64 = 2x32 DR slices at
                      tile_position (32*hh, 0)
  - attn*V:           OT[65, q] accumulated over 8 DR m-pair chunks
  - output proj:      Y[q, o] over 8 chunks of [65,2] (denominator row is
                      multiplied by zeroed pw rows)

PSUM can only be read by ACT and DVE, so exp(S/32), the K/Q/V fp8
converts, OT drains, and residual adds are greedily load-balanced
between those two engines at build time; Pool (gpsimd) handles all
SBUF-side work (denominator broadcast + CT scale, LN stats + finals).
exp on DVE uses the int8 bit trick:
  i8 = rne(S*8/(32*ln2) + 55.63), bitcast int8 -> fp8e4m3 ~= exp(S/32)
(the denominator sums the same approximated values -> consistent).

LayerNorm: device computes z = (y - mu)/(sigma_ddof1 + eps); the ln_a/ln_b
affine is applied on host. V is scaled x8 on host (pw /8) to keep CT
inside the fp8 normal range.
"""
import numpy as np
import ml_dtypes

import concourse.bass as bass
import concourse.mybir as mybir
import concourse.tile as tile
from concourse import bacc
from concourse.bass_utils import run_bass_kernel_spmd

F32 = mybir.dt.float32
F32R = mybir.dt.float32r
F8 = mybir.dt.float8e4
I8 = mybir.dt.int8
I32 = mybir.dt.int32
AF = mybir.ActivationFunctionType
ALU = mybir.AluOpType
PM = mybir.MatmulPerfMode
E4M3 = ml_dtypes.float8_e4m3

B, L, D = 4, 2048, 1024
H, DK = 16, 64
HALF = 1024            # query rows per core
TEMPER = 32.0          # sqrt(d_model)
G = 4                  # head groups of 4
LN_EPS = 1e-3
VSCALE = 8.0           # host scales w_vs by this, pw by 1/this
EXP_S1 = float(8.0 / (TEMPER * np.log(2.0)))
EXP_S2 = 56.0 - 0.37   # rne magic (calibrated on hw)
MAGIC_RCP = 0x7EF30000   # reciprocal seed; 1 Newton -> 0.26% max err
MAGIC_RSQ = 0x5F3759DF   # rsqrt seed; 2 Newtons -> 5e-6

_CACHE = {}


def build(iters=1):
    nc = bacc.Bacc(None, target_bir_lowering=False)
    qt8_d = nc.dram_tensor("qt8", [128, G * 2 * L], F8, kind="ExternalInput")
    wq8_d = nc.dram_tensor("wq8", [128, G * 2 * 1024], F8, kind="ExternalInput")
    wk8_d = nc.dram_tensor("wk8", [128, G * 2 * 1024], F8, kind="ExternalInput")
    wv8_d = nc.dram_tensor("wv8", [128, G * 2 * 1040], F8, kind="ExternalInput")
    pw8_d = nc.dram_tensor("pw8", [65, 8 * 2 * 1024], F8, kind="ExternalInput")
    qres_d = nc.dram_tensor("qres", [HALF, D], F32, kind="ExternalInput")
    out_d = nc.dram_tensor("out", [HALF, D], F32, kind="ExternalOutput")

    # build-time greedy ACT/DVE balancing (ns estimates incl. seq overhead)
    eng_ns = {"A": 0.0, "D": 0.0}

    def pick_ad(rows):
        ca = rows * 0.853 + 124.0
        cd = rows * 1.065 + 108.0
        if eng_ns["A"] + ca <= eng_ns["D"] + cd:
            eng_ns["A"] += ca
            return "A"
        eng_ns["D"] += cd
        return "D"

    def charge_d(rows):
        eng_ns["D"] += rows * 1.065 + 108.0

    def ad_copy(e, dst, src):
        if e == "A":
            nc.scalar.activation(dst, src, AF.Copy)
        else:
            nc.vector.tensor_copy(dst, src)

    with tile.TileContext(nc) as tc:
        with (
            tc.tile_pool(name="p1", bufs=1) as p1,
            tc.tile_pool(name="p2", bufs=2) as p2,
            tc.tile_pool(name="p3", bufs=6) as p3,
            tc.tile_pool(name="p4", bufs=4) as p4,
            tc.tile_pool(name="psS", bufs=2, space="PSUM") as psS,
            tc.tile_pool(name="psO", bufs=2, space="PSUM") as psO,
            tc.tile_pool(name="psA", bufs=2, space="PSUM") as psA,
        ):
            # ---- weight loads ----
            wk8_t = p1.tile([128, G, 2, 1024], F8, name="wk8_t")
            nc.sync.dma_start(wk8_t[:], wk8_d[:])
            qt8_t = p1.tile([128, G, 2, L], F8, name="qt8_t")
            for j in range(G):
                nc.scalar.dma_start(qt8_t[:, j, :, :],
                                    qt8_d[:, j * 2 * L:(j + 1) * 2 * L])
            wq8_t = p1.tile([128, G, 2, 1024], F8, name="wq8_t")
            nc.sync.dma_start(wq8_t[:], wq8_d[:])
            wv8_t = p1.tile([128, G, 2, 1040], F8, name="wv8_t")
            nc.sync.dma_start(wv8_t[:], wv8_d[:])
            pw8_t = p1.tile([65, 8, 2, 1024], F8, name="pw8_t")
            nc.sync.dma_start(pw8_t[:], pw8_d[:])

            for it in range(iters):
                sfx = f"i{it}"
                # per-group fp8 activation stores
                q8 = p1.tile([128, G, 2, HALF], F8, name=f"q8_{sfx}")
                k8 = p1.tile([128, G, 2, L], F8, name=f"k8_{sfx}")
                v8 = p1.tile([128, G, 8, 2, 320], F8, name=f"v8_{sfx}")
                ct8 = p1.tile([65, 8, 2, HALF], F8, name=f"ct8_{sfx}")
                # ones columns of v8 (col 0 of each head's 80-block);
                # the V convert copies only fill cols 1..64.
                for hh in range(4):
                    nc.gpsimd.memset(v8[:, :, :, :, hh * 80], 1.0)

                # layernorm stat tiles
                sums = p1.tile([128, 16], F32, name=f"sums_{sfx}")
                ssq16 = p1.tile([128, 16], F32, name=f"ssq16_{sfx}")
                mu8 = p1.tile([128, 8], F32, name=f"mu8_{sfx}")
                m28 = p1.tile([128, 8], F32, name=f"m28_{sfx}")
                ssq8 = p1.tile([128, 8], F32, name=f"ssq8_{sfx}")
                cs8 = p1.tile([128, 8], F32, name=f"cs8_{sfx}")
                var8 = p1.tile([128, 8], F32, name=f"var8_{sfx}")
                si8 = p1.tile([128, 8], I32, name=f"si8_{sfx}")
                a8 = p1.tile([128, 8], F32, name=f"a8_{sfx}")
                b8 = p1.tile([128, 8], F32, name=f"b8_{sfx}")
                rs8 = p1.tile([128, 8], F32, name=f"rs8_{sfx}")
                rr8 = p1.tile([128, 8], F32, name=f"rr8_{sfx}")
                rec8 = p1.tile([128, 8], F32, name=f"rec8_{sfx}")
                nmr8 = p1.tile([128, 8], F32, name=f"nmr8_{sfx}")
                y_ts = [p1.tile([128, D], F32, name=f"y_{sfx}_{qt}")
                        for qt in range(8)]

                def defer_weave(groups, lag=2):
                    """groups: list of (pe_closure, post_closure|None).
                    Weave so each post lands `lag` slots after its pe part."""
                    items = []
                    pend = []
                    for pe_f, post_f in groups:
                        items.append(pe_f)
                        pend.append(post_f)
                        if len(pend) > lag:
                            f = pend.pop(0)
                            if f is not None:
                                items.append(f)
                    for f in pend:
                        if f is not None:
                            items.append(f)
                    return items

                def kq_groups(g):
                    """K/Q projections for group g: (matmuls, convert)."""
                    groups = []

                    def kq(wt, dst, g, s, blk, nm):
                        hold = {}

                        def mms(hold=hold, g=g, s=s, blk=blk, nm=nm, wt=wt):
                            hold["t"] = psA.tile(
                                [128, 512], F32,
                                name=f"{nm}_{sfx}_{g}_{s}_{blk}", tag="acc")
                            for j in range(G):
                                nc.tensor.matmul(
                                    hold["t"][:],
                                    wt[:, j, :, g * 256 + s * 128:
                                       g * 256 + s * 128 + 128],
                                    qt8_t[:, j, :, blk * 512:(blk + 1) * 512],
                                    start=(j == 0), stop=(j == G - 1),
                                    perf_mode=PM.DoubleRow,
                                )

                        def cv(hold=hold, dst=dst):
                            ad_copy(pick_ad(512), dst, hold["t"][:])

                        return (mms, cv)

                    for s in range(2):
                        for mb in range(4):
                            groups.append(kq(
                                wk8_t, k8[:, g, s, mb * 512:(mb + 1) * 512],
                                g, s, mb, "kp"))
                    for s in range(2):
                        for qb in range(2):
                            groups.append(kq(
                                wq8_t, q8[:, g, s, qb * 512:(qb + 1) * 512],
                                g, s, qb, "qp"))
                    return defer_weave(groups)

                def v_groups(g):
                    """V projection for group g in m-tile order."""
                    groups = []
                    for mt in range(16):
                        hold = {}

                        def vmms(hold=hold, g=g, mt=mt):
                            hold["t"] = psA.tile(
                                [128, 512], F32,
                                name=f"vp_{sfx}_{g}_{mt}", tag="acc")
                            for j in range(G):
                                nc.tensor.matmul(
                                    hold["t"][:, 0:260],
                                    qt8_t[:, j, :, mt * 128:(mt + 1) * 128],
                                    wv8_t[:, j, :, g * 260:(g + 1) * 260],
                                    start=(j == 0), stop=(j == G - 1),
                                    perf_mode=PM.DoubleRow,
                                )

                        def vcv(hold=hold, g=g, mt=mt):
                            ad_copy(
                                pick_ad(256),
                                v8[:, g, mt // 2, mt % 2, :]
                                .rearrange("p (h f) -> p h f", h=4)[:, :, 1:65],
                                hold["t"][:, 0:260]
                                .rearrange("p (h f) -> p h f", h=4)[:, :, 1:65],
                            )

                        groups.append((vmms, vcv))
                    return defer_weave(groups, lag=0)

                def outproj_groups(qts):
                    """Output projection + y-add + squares as (pe, post)."""
                    groups = []
                    for qt in qts:
                        qr_hold = {}

                        def mk_mms(qt, oc, hold, qh):
                            def mms():
                                if oc == 0:
                                    qh["t"] = p4.tile([128, D], F32,
                                                      name=f"qr_{sfx}_{qt}",
                                                      tag="qr")
                                    nc.sync.dma_start(
                                        qh["t"][:],
                                        qres_d[qt * 128:(qt + 1) * 128, :])
                                hold["t"] = psA.tile(
                                    [128, 512], F32,
                                    name=f"yp_{sfx}_{qt}_{oc}", tag="acc")
                                jorder = list(range(8))
                                for n, j in enumerate(jorder):
                                    nc.tensor.matmul(
                                        hold["t"][:],
                                        ct8[:, j, :, qt * 128:(qt + 1) * 128],
                                        pw8_t[:, j, :, oc * 512:(oc + 1) * 512],
                                        start=(n == 0), stop=(n == 7),
                                        perf_mode=PM.DoubleRow,
                                    )
                            return mms

                        def mk_post(qt, oc, hold, qh):
                            def post():
                                nc.vector.scalar_tensor_tensor(
                                    y_ts[qt][:, oc * 512:(oc + 1) * 512],
                                    hold["t"][:], 1.0,
                                    qh["t"][:, oc * 512:(oc + 1) * 512],
                                    ALU.mult, ALU.add,
                                    accum_out=sums[:, 2 * qt + oc:
                                                   2 * qt + oc + 1])
                                charge_d(512)
                                sqt = p2.tile([128, 512], F32,
                                              name=f"sqt_{sfx}_{qt}_{oc}",
                                              tag="sqt")
                                yv = y_ts[qt][:, oc * 512:(oc + 1) * 512]
                                if pick_ad(512) == "A":
                                    nc.scalar.activation(
                                        sqt[:], yv, AF.Square,
                                        accum_out=ssq16[:, 2 * qt + oc:
                                                        2 * qt + oc + 1])
                                else:
                                    nc.vector.scalar_tensor_tensor(
                                        sqt[:], yv, 1.0, yv,
                                        ALU.mult, ALU.mult,
                                        accum_out=ssq16[:, 2 * qt + oc:
                                                        2 * qt + oc + 1])
                            return post

                        for oc in range(2):
                            hold = {}
                            groups.append((mk_mms(qt, oc, hold, qr_hold),
                                           mk_post(qt, oc, hold, qr_hold)))
                    return defer_weave(groups)

                def ln_chain(lo, hi):
                    """sigma chain + finals for q tiles [lo, hi)."""
                    cl = slice(lo, hi)
                    nc.gpsimd.tensor_tensor(ssq8[:, cl],
                                            ssq16[:, 2 * lo:2 * hi:2],
                                            ssq16[:, 2 * lo + 1:2 * hi:2],
                                            ALU.add)
                    nc.gpsimd.tensor_tensor(mu8[:, cl],
                                            sums[:, 2 * lo:2 * hi:2],
                                            sums[:, 2 * lo + 1:2 * hi:2],
                                            ALU.add)
                    nc.gpsimd.tensor_scalar(mu8[:, cl], mu8[:, cl], 1.0 / D,
                                            None, ALU.mult)
                    nc.gpsimd.tensor_tensor(m28[:, cl], mu8[:, cl], mu8[:, cl],
                                            ALU.mult)
                    nc.vector.scalar_tensor_tensor(cs8[:, cl], m28[:, cl],
                                                   -float(D), ssq8[:, cl],
                                                   ALU.mult, ALU.add)
                    nc.gpsimd.tensor_scalar(
                        var8[:, cl], cs8[:, cl],
                        1.0 / ((D - 1) * VSCALE * VSCALE), None, ALU.mult)
                    nc.vector.tensor_scalar(si8[:, cl],
                                            var8[:, cl].bitcast(I32), 1, None,
                                            ALU.arith_shift_right)
                    nc.vector.tensor_scalar(si8[:, cl], si8[:, cl], -1,
                                            MAGIC_RSQ, ALU.mult, ALU.add)
                    r_ap = si8[:, cl].bitcast(F32)
                    nc.gpsimd.tensor_tensor(a8[:, cl], r_ap, r_ap, ALU.mult)
                    nc.gpsimd.tensor_tensor(a8[:, cl], var8[:, cl], a8[:, cl],
                                            ALU.mult)
                    nc.gpsimd.tensor_scalar(a8[:, cl], a8[:, cl], -0.5, 1.5,
                                            ALU.mult, ALU.add)
                    nc.gpsimd.tensor_tensor(b8[:, cl], r_ap, a8[:, cl],
                                            ALU.mult)
                    nc.gpsimd.tensor_tensor(a8[:, cl], b8[:, cl], b8[:, cl],
                                            ALU.mult)
                    nc.gpsimd.tensor_tensor(a8[:, cl], var8[:, cl], a8[:, cl],
                                            ALU.mult)
                    nc.gpsimd.tensor_scalar(a8[:, cl], a8[:, cl], -0.5, 1.5,
                                            ALU.mult, ALU.add)
                    nc.gpsimd.tensor_tensor(rs8[:, cl], b8[:, cl], a8[:, cl],
                                            ALU.mult)
                    nc.gpsimd.tensor_tensor(rr8[:, cl], rs8[:, cl], rs8[:, cl],
                                            ALU.mult)
                    nc.vector.scalar_tensor_tensor(rec8[:, cl], rr8[:, cl],
                                                   -LN_EPS, rs8[:, cl],
                                                   ALU.mult, ALU.add)
                    nc.gpsimd.tensor_tensor(nmr8[:, cl], mu8[:, cl],
                                            rec8[:, cl], ALU.mult)
                    nc.gpsimd.tensor_scalar(nmr8[:, cl], nmr8[:, cl], -1.0,
                                            None, ALU.mult)
                    for qt in range(lo, hi):
                        o_t = p2.tile([128, D], F32, name=f"o_{sfx}_{qt}",
                                      tag="o")
                        if pick_ad(1024) == "A":
                            nc.scalar.activation(
                                o_t[:], y_ts[qt][:], AF.Identity,
                                bias=nmr8[:, qt:qt + 1],
                                scale=rec8[:, qt:qt + 1])
                        else:
                            nc.vector.tensor_scalar(
                                o_t[:], y_ts[qt][:], mu8[:, qt:qt + 1],
                                rec8[:, qt:qt + 1], ALU.subtract, ALU.mult)
                        dq = nc.sync if qt % 2 == 0 else nc.scalar
                        dq.dma_start(out_d[qt * 128:(qt + 1) * 128, :],
                                     o_t[:])

                def emit_head(g, qc, hh, filler, pace, pend):
                    qs = slice(qc * 512, (qc + 1) * 512)
                    p0 = 32 * hh
                    prow = slice(p0, p0 + 32)
                    ot = psO.tile([128, 512], F32,
                                  name=f"ot_{sfx}_{g}_{qc}_{hh}", tag="ot")
                    e8s = {}

                    def attnv(mip):
                        nc.tensor.matmul(
                            ot[0:65, :],
                            v8[:, g, mip, :, hh * 80:hh * 80 + 65],
                            e8s[mip][:],
                            start=(mip == 0), stop=(mip == 7),
                            perf_mode=PM.DoubleRow,
                        )

                    for mip in range(8):
                        sp = psS.tile([128, 1024], F32,
                                      name=f"sp_{sfx}_{g}_{qc}_{hh}_{mip}",
                                      tag="sc")
                        for k in range(2):
                            mi = 2 * mip + k
                            nc.tensor.matmul(
                                sp[:, k * 512:(k + 1) * 512],
                                k8[prow, g, :, mi * 128:(mi + 1) * 128],
                                q8[prow, g, :, qs],
                                start=True, stop=True,
                                perf_mode=PM.DoubleRow,
                                tile_position=(p0, 0),
                            )
                        e8 = p3.tile([128, 2, 512], F8,
                                     name=f"e8_{sfx}_{g}_{qc}_{hh}_{mip}",
                                     tag="e8")
                        e8s[mip] = e8
                        if pick_ad(1024) == "A":
                            nc.scalar.activation(
                                e8[:].rearrange("p s f -> p (s f)"),
                                sp[:], AF.Exp, scale=1.0 / TEMPER)
                        else:
                            nc.vector.tensor_scalar(
                                e8[:].bitcast(I8).rearrange("p s f -> p (s f)"),
                                sp[:], EXP_S1, EXP_S2, ALU.mult, ALU.add)
                        if mip == 2 and pend is not None:
                            pend()
                            pend = None
                        for _ in range(pace):
                            try:
                                next(filler)()
                            except StopIteration:
                                break
                        if mip >= 1:
                            attnv(mip - 1)
                    attnv(7)
                    if pend is not None:
                        pend()

                    def tail(g=g, qc=qc, hh=hh, ot=ot):
                        # drain -> recip(denominator) -> bcast -> scale
                        stage = p2.tile([65, 512], F32,
                                        name=f"st_{sfx}_{g}_{qc}_{hh}",
                                        tag="otst")
                        ad_copy(pick_ad(512), stage[:], ot[0:65, :])
                        rci = p2.tile([1, 512], I32,
                                      name=f"rci_{sfx}_{g}_{qc}_{hh}",
                                      tag="rci")
                        nc.gpsimd.tensor_scalar(rci[:],
                                                stage[0:1, :].bitcast(I32),
                                                -1, MAGIC_RCP,
                                                ALU.mult, ALU.add)
                        tt = p2.tile([1, 512], F32,
                                     name=f"tt_{sfx}_{g}_{qc}_{hh}", tag="tt")
                        nc.gpsimd.tensor_tensor(tt[:], stage[0:1, :],
                                                rci[:].bitcast(F32), ALU.mult)
                        nc.gpsimd.tensor_scalar(tt[:], tt[:], -1.0, 2.0,
                                                ALU.mult, ALU.add)
                        rc = p2.tile([1, 512], F32,
                                     name=f"rc_{sfx}_{g}_{qc}_{hh}", tag="rc")
                        nc.gpsimd.tensor_tensor(rc[:], rci[:].bitcast(F32),
                                                tt[:], ALU.mult)
                        rcb = p2.tile([65, 512], F32,
                                      name=f"rcb_{sfx}_{g}_{qc}_{hh}",
                                      tag="rcb")
                        nc.gpsimd.partition_broadcast(rcb[:], rc[:])
                        h = 4 * g + hh
                        nc.gpsimd.tensor_tensor(
                            ct8[:, h // 2, h % 2, qs], stage[:], rcb[:],
                            ALU.mult)

                    return tail

                # ---- emission: K/Q(0) upfront; attention filler = V(g)
                # then K/Q(g+1) projections / output proj ----
                for f in kq_groups(0):
                    f()
                pend = None
                for g in range(G):
                    fill_list = v_groups(g)
                    if g < G - 1:
                        fill_list = fill_list + kq_groups(g + 1)
                    filler = iter(fill_list)
                    pace = 1
                    for qc in range(2):
                        if g == G - 1 and qc == 1:
                            if pend is not None:
                                pend()
                                pend = None
                            for f in filler:
                                f()
                            fill_list = outproj_groups(range(4))
                            filler = iter(fill_list)
                            pace = 2
                        for hh in range(4):
                            hp = 4 if (qc == 0 and hh == 0) else pace
                            pend = emit_head(g, qc, hh, filler, hp, pend)
                    for f in filler:
                        f()
                if pend is not None:
                    pend()

                # ---- tail: finals for qt 0-3 overlap outproj qt 4-7 ----
                ln_chain(0, 4)
                for f in outproj_groups(range(4, 8)):
                    f()
                ln_chain(4, 8)

    nc.compile()
    return nc


def _get_nc():
    if "nc" not in _CACHE:
        _CACHE["nc"] = build()
    return _CACHE["nc"]


def _prep_shared(w_qs, w_ks, w_vs, proj_w):
    """fp8 weight layouts: rows d -> [p, j, s] with d = 256j + 128s + p."""
    def dsplit(a):  # [1024, N] -> [128, 4*2*N]
        n = a.shape[1]
        return np.ascontiguousarray(
            a.reshape(G, 2, 128, n).transpose(2, 0, 1, 3).reshape(128, -1)
        )

    # wq/wk cols: g*256 + (dk//32)*128 + hh*32 + dk%32  <- head 4g+hh
    wq = np.empty((D, H * DK), dtype=np.float32)
    wk = np.empty((D, H * DK), dtype=np.float32)
    for g in range(G):
        for s in range(2):
            for hh in range(4):
                c0 = g * 256 + s * 128 + hh * 32
                wq[:, c0:c0 + 32] = w_qs[4 * g + hh, :, 32 * s:32 * s + 32]
                wk[:, c0:c0 + 32] = w_ks[4 * g + hh, :, 32 * s:32 * s + 32]
    # wv cols: g*260 + hh*65 + (1+dv); col hh*65 is the ones slot
    wv = np.zeros((D, G * 4 * 65), dtype=np.float32)
    for g in range(G):
        for hh in range(4):
            c0 = g * 260 + hh * 65
            wv[:, c0 + 1:c0 + 65] = w_vs[4 * g + hh] * VSCALE
    # pw8 [65, 8, 2, 1024]: row p=0 zero (denominator slot), p=1+dv maps
    # to concat row (2j+s)*64+dv of proj_w.T
    pwT = proj_w.T.astype(np.float32)  # [c, o]
    pw8 = np.zeros((65, 8, 2, D), dtype=np.float32)
    for j in range(8):
        for s in range(2):
            h = 2 * j + s
            pw8[1:65, j, s, :] = pwT[h * 64:(h + 1) * 64, :]
    pw8 = pw8.reshape(65, -1)
    wq8 = dsplit(wq).astype(E4M3)
    wk8 = dsplit(wk).astype(E4M3)
    wv8 = dsplit(wv).astype(E4M3)
    pw8 = np.ascontiguousarray(pw8).astype(E4M3)
    return wq8, wk8, wv8, pw8


def kernel(q, w_qs, w_ks, w_vs, proj_w, proj_b, ln_a, ln_b, **kw):
    q = np.asarray(q, dtype=np.float32)
    w_qs = np.asarray(w_qs, dtype=np.float32)
    w_ks = np.asarray(w_ks, dtype=np.float32)
    w_vs = np.asarray(w_vs, dtype=np.float32)
    proj_w = np.asarray(proj_w, dtype=np.float32)
    proj_b = np.asarray(proj_b, dtype=np.float32)
    ln_a = np.asarray(ln_a, dtype=np.float32)
    ln_b = np.asarray(ln_b, dtype=np.float32)

    wq8, wk8, wv8, pw8 = _prep_shared(w_qs, w_ks, w_vs, proj_w)

    in_maps = []
    for c in range(8):
        b, half = c // 2, c % 2
        qbT = q[b].T  # [D, L]
        qcat = np.concatenate(
            [qbT[:, half * HALF:(half + 1) * HALF],
             qbT[:, (1 - half) * HALF:(2 - half) * HALF]], axis=1)
        qt8 = np.ascontiguousarray(
            qcat.reshape(G, 2, 128, L).transpose(2, 0, 1, 3).reshape(128, -1)
        ).astype(E4M3)
        qres_c = np.ascontiguousarray(
            (q[b, half * HALF:(half + 1) * HALF, :] + proj_b[None, :]) * VSCALE)
        in_maps.append({
            "qt8": qt8, "qres": qres_c,
            "wq8": wq8, "wk8": wk8, "wv8": wv8, "pw8": pw8,
        })

    nc = _get_nc()
    res = run_bass_kernel_spmd(nc, in_maps, core_ids=list(range(8))).results

    out = np.empty((B, L, D), dtype=np.float32)
    for c in range(8):
        b, half = c // 2, c % 2
        out[b, half * HALF:(half + 1) * HALF, :] = res[c]["out"]
    # ln affine on host
    out = out * (ln_a[None, None, :] / VSCALE) + ln_b[None, None, :]
    return out


# revision 29
# speedup vs baseline: 1.0197x; 1.0197x over previous
"""MultiHeadAttention TRN2 Bass kernel (8 NeuronCores), fp8 DoubleRow edition.

Sharding: core c = (batch b = c//2, query-half = c%2). Each core computes
K/V for its full batch (2048 keys) and attention + output projection + LN
for its 1024 query rows. No collectives; host gathers per-core outputs.

All heavy matmuls run in fp8e4m3 with MatmulPerfMode.DoubleRow (0.5 PE
cycles per output row, 256-wide contraction per instruction):
  - Q/K projections:  out[4head*32dk half, q|m] over D=1024 (4 DR chunks)
  - V projection:     Vaug[m, 4head*65] (col 0 of each head = ones so the
                      softmax denominator lands at partition 0 of OT)
  - scores:           S[m, q] per head, dk=64 = 2x32 DR slices at
                      tile_position (32*hh, 0)
  - attn*V:           OT[65, q] accumulated over 8 DR m-pair chunks
  - output proj:      Y[q, o] over 8 chunks of [65,2] (denominator row is
                      multiplied by zeroed pw rows)

PSUM can only be read by ACT and DVE, so exp(S/32), the K/Q/V fp8
converts, OT drains, and residual adds are greedily load-balanced
between those two engines at build time; Pool (gpsimd) handles all
SBUF-side work (denominator broadcast + CT scale, LN stats + finals).
exp on DVE uses the int8 bit trick:
  i8 = rne(S*8/(32*ln2) + 55.63), bitcast int8 -> fp8e4m3 ~= exp(S/32)
(the denominator sums the same approximated values -> consistent).

LayerNorm: device computes z = (y - mu)/(sigma_ddof1 + eps); the ln_a/ln_b
affine is applied on host. V is scaled x8 on host (pw /8) to keep CT
inside the fp8 normal range.
"""
import numpy as np
import ml_dtypes

import concourse.bass as bass
import concourse.mybir as mybir
import concourse.tile as tile
from concourse import bacc
from concourse.bass_utils import run_bass_kernel_spmd

F32 = mybir.dt.float32
F32R = mybir.dt.float32r
F8 = mybir.dt.float8e4
I8 = mybir.dt.int8
I32 = mybir.dt.int32
AF = mybir.ActivationFunctionType
ALU = mybir.AluOpType
PM = mybir.MatmulPerfMode
E4M3 = ml_dtypes.float8_e4m3

B, L, D = 4, 2048, 1024
H, DK = 16, 64
HALF = 1024            # query rows per core
TEMPER = 32.0          # sqrt(d_model)
G = 4                  # head groups of 4
LN_EPS = 1e-3
VSCALE = 8.0           # host scales w_vs by this, pw by 1/this
EXP_S1 = float(8.0 / (TEMPER * np.log(2.0)))
EXP_S2 = 56.0 - 0.37   # rne magic (calibrated on hw)
MAGIC_RCP = 0x7EF30000   # reciprocal seed; 1 Newton -> 0.26% max err
MAGIC_RSQ = 0x5F3759DF   # rsqrt seed; 2 Newtons -> 5e-6

_CACHE = {}


def build(iters=1):
    nc = bacc.Bacc(None, target_bir_lowering=False)
    qt8_d = nc.dram_tensor("qt8", [128, G * 2 * L], F8, kind="ExternalInput")
    wq8_d = nc.dram_tensor("wq8", [128, G * 2 * 1024], F8, kind="ExternalInput")
    wk8_d = nc.dram_tensor("wk8", [128, G * 2 * 1024], F8, kind="ExternalInput")
    wv8_d = nc.dram_tensor("wv8", [128, G * 2 * 1040], F8, kind="ExternalInput")
    pw8_d = nc.dram_tensor("pw8", [65, 8 * 2 * 1024], F8, kind="ExternalInput")
    qres_d = nc.dram_tensor("qres", [HALF, D], F32, kind="ExternalInput")
    out_d = nc.dram_tensor("out", [HALF, D], F32, kind="ExternalOutput")

    # build-time greedy ACT/DVE balancing (ns estimates incl. seq overhead)
    eng_ns = {"A": 0.0, "D": 0.0}

    def pick_ad(rows):
        ca = rows * 0.853 + 124.0
        cd = rows * 1.065 + 108.0
        if eng_ns["A"] + ca <= eng_ns["D"] + cd:
            eng_ns["A"] += ca
            return "A"
        eng_ns["D"] += cd
        return "D"

    def charge_d(rows):
        eng_ns["D"] += rows * 1.065 + 108.0

    def ad_copy(e, dst, src):
        if e == "A":
            nc.scalar.activation(dst, src, AF.Copy)
        else:
            nc.vector.tensor_copy(dst, src)

    with tile.TileContext(nc) as tc:
        with (
            tc.tile_pool(name="p1", bufs=1) as p1,
            tc.tile_pool(name="p2", bufs=2) as p2,
            tc.tile_pool(name="p3", bufs=6) as p3,
            tc.tile_pool(name="p4", bufs=4) as p4,
            tc.tile_pool(name="psS", bufs=2, space="PSUM") as psS,
            tc.tile_pool(name="psO", bufs=2, space="PSUM") as psO,
            tc.tile_pool(name="psA", bufs=2, space="PSUM") as psA,
        ):
            # ---- weight loads ----
            wk8_t = p1.tile([128, G, 2, 1024], F8, name="wk8_t")
            nc.sync.dma_start(wk8_t[:], wk8_d[:])
            qt8_t = p1.tile([128, G, 2, L], F8, name="qt8_t")
            for j in range(G):
                nc.scalar.dma_start(qt8_t[:, j, :, :],
                                    qt8_d[:, j * 2 * L:(j + 1) * 2 * L])
            wq8_t = p1.tile([128, G, 2, 1024], F8, name="wq8_t")
            nc.sync.dma_start(wq8_t[:], wq8_d[:])
            wv8_t = p1.tile([128, G, 2, 1040], F8, name="wv8_t")
            nc.sync.dma_start(wv8_t[:], wv8_d[:])
            pw8_t = p1.tile([65, 8, 2, 1024], F8, name="pw8_t")
            nc.sync.dma_start(pw8_t[:], pw8_d[:])

            for it in range(iters):
                sfx = f"i{it}"
                # per-group fp8 activation stores
                q8 = p1.tile([128, G, 2, HALF], F8, name=f"q8_{sfx}")
                k8 = p1.tile([128, G, 2, L], F8, name=f"k8_{sfx}")
                v8 = p1.tile([128, G, 8, 2, 320], F8, name=f"v8_{sfx}")
                ct8 = p1.tile([65, 8, 2, HALF], F8, name=f"ct8_{sfx}")
                # ones columns of v8 (col 0 of each head's 80-block);
                # the V convert copies only fill cols 1..64.
                for hh in range(4):
                    nc.gpsimd.memset(v8[:, :, :, :, hh * 80], 1.0)

                # layernorm stat tiles
                sums = p1.tile([128, 16], F32, name=f"sums_{sfx}")
                ssq16 = p1.tile([128, 16], F32, name=f"ssq16_{sfx}")
                mu8 = p1.tile([128, 8], F32, name=f"mu8_{sfx}")
                m28 = p1.tile([128, 8], F32, name=f"m28_{sfx}")
                ssq8 = p1.tile([128, 8], F32, name=f"ssq8_{sfx}")
                cs8 = p1.tile([128, 8], F32, name=f"cs8_{sfx}")
                var8 = p1.tile([128, 8], F32, name=f"var8_{sfx}")
                si8 = p1.tile([128, 8], I32, name=f"si8_{sfx}")
                a8 = p1.tile([128, 8], F32, name=f"a8_{sfx}")
                b8 = p1.tile([128, 8], F32, name=f"b8_{sfx}")
                rs8 = p1.tile([128, 8], F32, name=f"rs8_{sfx}")
                rr8 = p1.tile([128, 8], F32, name=f"rr8_{sfx}")
                rec8 = p1.tile([128, 8], F32, name=f"rec8_{sfx}")
                nmr8 = p1.tile([128, 8], F32, name=f"nmr8_{sfx}")
                y_ts = [p1.tile([128, D], F32, name=f"y_{sfx}_{qt}")
                        for qt in range(8)]

                def defer_weave(groups, lag=2):
                    """groups: list of (pe_closure, post_closure|None).
                    Weave so each post lands `lag` slots after its pe part."""
                    items = []
                    pend = []
                    for pe_f, post_f in groups:
                        items.append(pe_f)
                        pend.append(post_f)
                        if len(pend) > lag:
                            f = pend.pop(0)
                            if f is not None:
                                items.append(f)
                    for f in pend:
                        if f is not None:
                            items.append(f)
                    return items

                def kq_groups(g):
                    """K/Q projections for group g: (matmuls, convert)."""
                    groups = []

                    def kq(wt, dst, g, s, blk, nm):
                        hold = {}

                        def mms(hold=hold, g=g, s=s, blk=blk, nm=nm, wt=wt):
                            hold["t"] = psA.tile(
                                [128, 512], F32,
                                name=f"{nm}_{sfx}_{g}_{s}_{blk}", tag="acc")
                            for j in range(G):
                                nc.tensor.matmul(
                                    hold["t"][:],
                                    wt[:, j, :, g * 256 + s * 128:
                                       g * 256 + s * 128 + 128],
                                    qt8_t[:, j, :, blk * 512:(blk + 1) * 512],
                                    start=(j == 0), stop=(j == G - 1),
                                    perf_mode=PM.DoubleRow,
                                )

                        def cv(hold=hold, dst=dst):
                            ad_copy(pick_ad(512), dst, hold["t"][:])

                        return (mms, cv)

                    for s in range(2):
                        for mb in range(4):
                            groups.append(kq(
                                wk8_t, k8[:, g, s, mb * 512:(mb + 1) * 512],
                                g, s, mb, "kp"))
                    for s in range(2):
                        for qb in range(2):
                            groups.append(kq(
                                wq8_t, q8[:, g, s, qb * 512:(qb + 1) * 512],
                                g, s, qb, "qp"))
                    return defer_weave(groups)

                def v_groups(g):
                    """V projection for group g in m-tile order."""
                    groups = []
                    for mt in range(16):
                        hold = {}

                        def vmms(hold=hold, g=g, mt=mt):
                            hold["t"] = psA.tile(
                                [128, 512], F32,
                                name=f"vp_{sfx}_{g}_{mt}", tag="acc")
                            for j in range(G):
                                nc.tensor.matmul(
                                    hold["t"][:, 0:260],
                                    qt8_t[:, j, :, mt * 128:(mt + 1) * 128],
                                    wv8_t[:, j, :, g * 260:(g + 1) * 260],
                                    start=(j == 0), stop=(j == G - 1),
                                    perf_mode=PM.DoubleRow,
                                )

                        def vcv(hold=hold, g=g, mt=mt):
                            ad_copy(
                                pick_ad(256),
                                v8[:, g, mt // 2, mt % 2, :]
                                .rearrange("p (h f) -> p h f", h=4)[:, :, 1:65],
                                hold["t"][:, 0:260]
                                .rearrange("p (h f) -> p h f", h=4)[:, :, 1:65],
                            )

                        groups.append((vmms, vcv))
                    return defer_weave(groups, lag=0)

                def outproj_groups(qts):
                    """Output projection + y-add + squares as (pe, post)."""
                    groups = []
                    for qt in qts:
                        qr_hold = {}

                        def mk_mms(qt, oc, hold, qh):
                            def mms():
                                if oc == 0:
                                    qh["t"] = p4.tile([128, D], F32,
                                                      name=f"qr_{sfx}_{qt}",
                                                      tag="qr")
                                    nc.sync.dma_start(
                                        qh["t"][:],
                                        qres_d[qt * 128:(qt + 1) * 128, :])
                                hold["t"] = psA.tile(
                                    [128, 512], F32,
                                    name=f"yp_{sfx}_{qt}_{oc}", tag="acc")
                                jorder = list(range(8))
                                for n, j in enumerate(jorder):
                                    nc.tensor.matmul(
                                        hold["t"][:],
                                        ct8[:, j, :, qt * 128:(qt + 1) * 128],
                                        pw8_t[:, j, :, oc * 512:(oc + 1) * 512],
                                        start=(n == 0), stop=(n == 7),
                                        perf_mode=PM.DoubleRow,
                                    )
                            return mms

                        def mk_post(qt, oc, hold, qh):
                            def post():
                                nc.vector.scalar_tensor_tensor(
                                    y_ts[qt][:, oc * 512:(oc + 1) * 512],
                                    hold["t"][:], 1.0,
                                    qh["t"][:, oc * 512:(oc + 1) * 512],
                                    ALU.mult, ALU.add,
                                    accum_out=sums[:, 2 * qt + oc:
                                                   2 * qt + oc + 1])
                                charge_d(512)
                                sqt = p2.tile([128, 512], F32,
                                              name=f"sqt_{sfx}_{qt}_{oc}",
                                              tag="sqt")
                                yv = y_ts[qt][:, oc * 512:(oc + 1) * 512]
                                if pick_ad(512) == "A":
                                    nc.scalar.activation(
                                        sqt[:], yv, AF.Square,
                                        accum_out=ssq16[:, 2 * qt + oc:
                                                        2 * qt + oc + 1])
                                else:
                                    nc.vector.scalar_tensor_tensor(
                                        sqt[:], yv, 1.0, yv,
                                        ALU.mult, ALU.mult,
                                        accum_out=ssq16[:, 2 * qt + oc:
                                                        2 * qt + oc + 1])
                            return post

                        for oc in range(2):
                            hold = {}
                            groups.append((mk_mms(qt, oc, hold, qr_hold),
                                           mk_post(qt, oc, hold, qr_hold)))
                    return defer_weave(groups)

                def ln_chain(lo, hi):
                    """sigma chain + finals for q tiles [lo, hi)."""
                    cl = slice(lo, hi)
                    nc.gpsimd.tensor_tensor(ssq8[:, cl],
                                            ssq16[:, 2 * lo:2 * hi:2],
                                            ssq16[:, 2 * lo + 1:2 * hi:2],
                                            ALU.add)
                    nc.gpsimd.tensor_tensor(mu8[:, cl],
                                            sums[:, 2 * lo:2 * hi:2],
                                            sums[:, 2 * lo + 1:2 * hi:2],
                                            ALU.add)
                    nc.gpsimd.tensor_scalar(mu8[:, cl], mu8[:, cl], 1.0 / D,
                                            None, ALU.mult)
                    nc.gpsimd.tensor_tensor(m28[:, cl], mu8[:, cl], mu8[:, cl],
                                            ALU.mult)
                    nc.vector.scalar_tensor_tensor(cs8[:, cl], m28[:, cl],
                                                   -float(D), ssq8[:, cl],
                                                   ALU.mult, ALU.add)
                    nc.gpsimd.tensor_scalar(
                        var8[:, cl], cs8[:, cl],
                        1.0 / ((D - 1) * VSCALE * VSCALE), None, ALU.mult)
                    nc.vector.tensor_scalar(si8[:, cl],
                                            var8[:, cl].bitcast(I32), 1, None,
                                            ALU.arith_shift_right)
                    nc.vector.tensor_scalar(si8[:, cl], si8[:, cl], -1,
                                            MAGIC_RSQ, ALU.mult, ALU.add)
                    r_ap = si8[:, cl].bitcast(F32)
                    nc.gpsimd.tensor_tensor(a8[:, cl], r_ap, r_ap, ALU.mult)
                    nc.gpsimd.tensor_tensor(a8[:, cl], var8[:, cl], a8[:, cl],
                                            ALU.mult)
                    nc.gpsimd.tensor_scalar(a8[:, cl], a8[:, cl], -0.5, 1.5,
                                            ALU.mult, ALU.add)
                    nc.gpsimd.tensor_tensor(b8[:, cl], r_ap, a8[:, cl],
                                            ALU.mult)
                    nc.gpsimd.tensor_tensor(a8[:, cl], b8[:, cl], b8[:, cl],
                                            ALU.mult)
                    nc.gpsimd.tensor_tensor(a8[:, cl], var8[:, cl], a8[:, cl],
                                            ALU.mult)
                    nc.gpsimd.tensor_scalar(a8[:, cl], a8[:, cl], -0.5, 1.5,
                                            ALU.mult, ALU.add)
                    nc.gpsimd.tensor_tensor(rs8[:, cl], b8[:, cl], a8[:, cl],
                                            ALU.mult)
                    nc.gpsimd.tensor_tensor(rr8[:, cl], rs8[:, cl], rs8[:, cl],
                                            ALU.mult)
                    nc.vector.scalar_tensor_tensor(rec8[:, cl], rr8[:, cl],
                                                   -LN_EPS, rs8[:, cl],
                                                   ALU.mult, ALU.add)
                    nc.gpsimd.tensor_tensor(nmr8[:, cl], mu8[:, cl],
                                            rec8[:, cl], ALU.mult)
                    nc.gpsimd.tensor_scalar(nmr8[:, cl], nmr8[:, cl], -1.0,
                                            None, ALU.mult)
                    for qt in range(lo, hi):
                        o_t = p2.tile([128, D], F32, name=f"o_{sfx}_{qt}",
                                      tag="o")
                        if pick_ad(1024) == "A":
                            nc.scalar.activation(
                                o_t[:], y_ts[qt][:], AF.Identity,
                                bias=nmr8[:, qt:qt + 1],
                                scale=rec8[:, qt:qt + 1])
                        else:
                            nc.vector.tensor_scalar(
                                o_t[:], y_ts[qt][:], mu8[:, qt:qt + 1],
                                rec8[:, qt:qt + 1], ALU.subtract, ALU.mult)
                        dq = nc.sync if qt % 2 == 0 else nc.scalar
                        dq.dma_start(out_d[qt * 128:(qt + 1) * 128, :],
                                     o_t[:])

                def emit_head(g, qc, hh, filler, pace, pend):
                    qs = slice(qc * 512, (qc + 1) * 512)
                    p0 = 32 * hh
                    prow = slice(p0, p0 + 32)
                    ot = psO.tile([128, 512], F32,
                                  name=f"ot_{sfx}_{g}_{qc}_{hh}", tag="ot")
                    e8s = {}

                    def attnv(mip):
                        nc.tensor.matmul(
                            ot[0:65, :],
                            v8[:, g, mip, :, hh * 80:hh * 80 + 65],
                            e8s[mip][:],
                            start=(mip == 0), stop=(mip == 7),
                            perf_mode=PM.DoubleRow,
                        )

                    for mip in range(8):
                        sp = psS.tile([128, 1024], F32,
                                      name=f"sp_{sfx}_{g}_{qc}_{hh}_{mip}",
                                      tag="sc")
                        for k in range(2):
                            mi = 2 * mip + k
                            nc.tensor.matmul(
                                sp[:, k * 512:(k + 1) * 512],
                                k8[prow, g, :, mi * 128:(mi + 1) * 128],
                                q8[prow, g, :, qs],
                                start=True, stop=True,
                                perf_mode=PM.DoubleRow,
                                tile_position=(p0, 0),
                            )
                        e8 = p3.tile([128, 2, 512], F8,
                                     name=f"e8_{sfx}_{g}_{qc}_{hh}_{mip}",
                                     tag="e8")
                        e8s[mip] = e8
                        if pick_ad(1024) == "A":
                            nc.scalar.activation(
                                e8[:].rearrange("p s f -> p (s f)"),
                                sp[:], AF.Exp, scale=1.0 / TEMPER)
                        else:
                            nc.vector.tensor_scalar(
                                e8[:].bitcast(I8).rearrange("p s f -> p (s f)"),
                                sp[:], EXP_S1, EXP_S2, ALU.mult, ALU.add)
                        if mip == 1 and pend is not None:
                            pend()
                            pend = None
                        for _ in range(pace):
                            try:
                                next(filler)()
                            except StopIteration:
                                break
                        if mip >= 1:
                            attnv(mip - 1)
                    attnv(7)
                    if pend is not None:
                        pend()

                    def tail(g=g, qc=qc, hh=hh, ot=ot):
                        # drain -> recip(denominator) -> bcast -> scale
                        stage = p2.tile([65, 512], F32,
                                        name=f"st_{sfx}_{g}_{qc}_{hh}",
                                        tag="otst")
                        ad_copy(pick_ad(512), stage[:], ot[0:65, :])
                        rci = p2.tile([1, 512], I32,
                                      name=f"rci_{sfx}_{g}_{qc}_{hh}",
                                      tag="rci")
                        nc.gpsimd.tensor_scalar(rci[:],
                                                stage[0:1, :].bitcast(I32),
                                                -1, MAGIC_RCP,
                                                ALU.mult, ALU.add)
                        tt = p2.tile([1, 512], F32,
                                     name=f"tt_{sfx}_{g}_{qc}_{hh}", tag="tt")
                        nc.gpsimd.tensor_tensor(tt[:], stage[0:1, :],
                                                rci[:].bitcast(F32), ALU.mult)
                        nc.gpsimd.tensor_scalar(tt[:], tt[:], -1.0, 2.0,
                                                ALU.mult, ALU.add)
                        rc = p2.tile([1, 512], F32,
                                     name=f"rc_{sfx}_{g}_{qc}_{hh}", tag="rc")
                        nc.gpsimd.tensor_tensor(rc[:], rci[:].bitcast(F32),
                                                tt[:], ALU.mult)
                        rcb = p2.tile([65, 512], F32,
                                      name=f"rcb_{sfx}_{g}_{qc}_{hh}",
                                      tag="rcb")
                        nc.gpsimd.partition_broadcast(rcb[:], rc[:])
                        h = 4 * g + hh
                        nc.gpsimd.tensor_tensor(
                            ct8[:, h // 2, h % 2, qs], stage[:], rcb[:],
                            ALU.mult)

                    return tail

                # ---- emission: K/Q(0) upfront; attention filler = V(g)
                # then K/Q(g+1) projections / output proj ----
                for f in kq_groups(0):
                    f()
                pend = None
                for g in range(G):
                    fill_list = v_groups(g)
                    if g < G - 1:
                        fill_list = fill_list + kq_groups(g + 1)
                    filler = iter(fill_list)
                    pace = 1
                    for qc in range(2):
                        if g == G - 1 and qc == 1:
                            if pend is not None:
                                pend()
                                pend = None
                            for f in filler:
                                f()
                            fill_list = outproj_groups(range(4))
                            filler = iter(fill_list)
                            pace = 2
                        for hh in range(4):
                            hp = 4 if (qc == 0 and hh == 0) else pace
                            pend = emit_head(g, qc, hh, filler, hp, pend)
                    for f in filler:
                        f()
                if pend is not None:
                    pend()

                # ---- tail: finals for qt 0-3 overlap outproj qt 4-7 ----
                ln_chain(0, 4)
                for f in outproj_groups(range(4, 8)):
                    f()
                ln_chain(4, 8)

    nc.compile()
    return nc


def _get_nc():
    if "nc" not in _CACHE:
        _CACHE["nc"] = build()
    return _CACHE["nc"]


def _prep_shared(w_qs, w_ks, w_vs, proj_w):
    """fp8 weight layouts: rows d -> [p, j, s] with d = 256j + 128s + p."""
    def dsplit(a):  # [1024, N] -> [128, 4*2*N]
        n = a.shape[1]
        return np.ascontiguousarray(
            a.reshape(G, 2, 128, n).transpose(2, 0, 1, 3).reshape(128, -1)
        )

    # wq/wk cols: g*256 + (dk//32)*128 + hh*32 + dk%32  <- head 4g+hh
    wq = np.empty((D, H * DK), dtype=np.float32)
    wk = np.empty((D, H * DK), dtype=np.float32)
    for g in range(G):
        for s in range(2):
            for hh in range(4):
                c0 = g * 256 + s * 128 + hh * 32
                wq[:, c0:c0 + 32] = w_qs[4 * g + hh, :, 32 * s:32 * s + 32]
                wk[:, c0:c0 + 32] = w_ks[4 * g + hh, :, 32 * s:32 * s + 32]
    # wv cols: g*260 + hh*65 + (1+dv); col hh*65 is the ones slot
    wv = np.zeros((D, G * 4 * 65), dtype=np.float32)
    for g in range(G):
        for hh in range(4):
            c0 = g * 260 + hh * 65
            wv[:, c0 + 1:c0 + 65] = w_vs[4 * g + hh] * VSCALE
    # pw8 [65, 8, 2, 1024]: row p=0 zero (denominator slot), p=1+dv maps
    # to concat row (2j+s)*64+dv of proj_w.T
    pwT = proj_w.T.astype(np.float32)  # [c, o]
    pw8 = np.zeros((65, 8, 2, D), dtype=np.float32)
    for j in range(8):
        for s in range(2):
            h = 2 * j + s
            pw8[1:65, j, s, :] = pwT[h * 64:(h + 1) * 64, :]
    pw8 = pw8.reshape(65, -1)
    wq8 = dsplit(wq).astype(E4M3)
    wk8 = dsplit(wk).astype(E4M3)
    wv8 = dsplit(wv).astype(E4M3)
    pw8 = np.ascontiguousarray(pw8).astype(E4M3)
    return wq8, wk8, wv8, pw8


def kernel(q, w_qs, w_ks, w_vs, proj_w, proj_b, ln_a, ln_b, **kw):
    q = np.asarray(q, dtype=np.float32)
    w_qs = np.asarray(w_qs, dtype=np.float32)
    w_ks = np.asarray(w_ks, dtype=np.float32)
    w_vs = np.asarray(w_vs, dtype=np.float32)
    proj_w = np.asarray(proj_w, dtype=np.float32)
    proj_b = np.asarray(proj_b, dtype=np.float32)
    ln_a = np.asarray(ln_a, dtype=np.float32)
    ln_b = np.asarray(ln_b, dtype=np.float32)

    wq8, wk8, wv8, pw8 = _prep_shared(w_qs, w_ks, w_vs, proj_w)

    in_maps = []
    for c in range(8):
        b, half = c // 2, c % 2
        qbT = q[b].T  # [D, L]
        qcat = np.concatenate(
            [qbT[:, half * HALF:(half + 1) * HALF],
             qbT[:, (1 - half) * HALF:(2 - half) * HALF]], axis=1)
        qt8 = np.ascontiguousarray(
            qcat.reshape(G, 2, 128, L).transpose(2, 0, 1, 3).reshape(128, -1)
        ).astype(E4M3)
        qres_c = np.ascontiguousarray(
            (q[b, half * HALF:(half + 1) * HALF, :] + proj_b[None, :]) * VSCALE)
        in_maps.append({
            "qt8": qt8, "qres": qres_c,
            "wq8": wq8, "wk8": wk8, "wv8": wv8, "pw8": pw8,
        })

    nc = _get_nc()
    res = run_bass_kernel_spmd(nc, in_maps, core_ids=list(range(8))).results

    out = np.empty((B, L, D), dtype=np.float32)
    for c in range(8):
        b, half = c // 2, c % 2
        out[b, half * HALF:(half + 1) * HALF, :] = res[c]["out"]
    # ln affine on host
    out = out * (ln_a[None, None, :] / VSCALE) + ln_b[None, None, :]
    return out


# revision 30
# speedup vs baseline: 1.0197x; 1.0000x over previous
"""MultiHeadAttention TRN2 Bass kernel (8 NeuronCores), fp8 DoubleRow edition.

Sharding: core c = (batch b = c//2, query-half = c%2). Each core computes
K/V for its full batch (2048 keys) and attention + output projection + LN
for its 1024 query rows. No collectives; host gathers per-core outputs.

All heavy matmuls run in fp8e4m3 with MatmulPerfMode.DoubleRow (0.5 PE
cycles per output row, 256-wide contraction per instruction):
  - Q/K projections:  out[4head*32dk half, q|m] over D=1024 (4 DR chunks)
  - V projection:     Vaug[m, 4head*65] (col 0 of each head = ones so the
                      softmax denominator lands at partition 0 of OT)
  - scores:           S[m, q] per head, dk=64 = 2x32 DR slices at
                      tile_position (32*hh, 0)
  - attn*V:           OT[65, q] accumulated over 8 DR m-pair chunks
  - output proj:      Y[q, o] over 8 chunks of [65,2] (denominator row is
                      multiplied by zeroed pw rows)

PSUM can only be read by ACT and DVE, so exp(S/32), the K/Q/V fp8
converts, OT drains, and residual adds are greedily load-balanced
between those two engines at build time; Pool (gpsimd) handles all
SBUF-side work (denominator broadcast + CT scale, LN stats + finals).
exp on DVE uses the int8 bit trick:
  i8 = rne(S*8/(32*ln2) + 55.63), bitcast int8 -> fp8e4m3 ~= exp(S/32)
(the denominator sums the same approximated values -> consistent).

LayerNorm: device computes z = (y - mu)/(sigma_ddof1 + eps); the ln_a/ln_b
affine is applied on host. V is scaled x8 on host (pw /8) to keep CT
inside the fp8 normal range.
"""
import numpy as np
import ml_dtypes

import concourse.bass as bass
import concourse.mybir as mybir
import concourse.tile as tile
from concourse import bacc
from concourse.bass_utils import run_bass_kernel_spmd

F32 = mybir.dt.float32
F32R = mybir.dt.float32r
F8 = mybir.dt.float8e4
I8 = mybir.dt.int8
I32 = mybir.dt.int32
AF = mybir.ActivationFunctionType
ALU = mybir.AluOpType
PM = mybir.MatmulPerfMode
E4M3 = ml_dtypes.float8_e4m3

B, L, D = 4, 2048, 1024
H, DK = 16, 64
HALF = 1024            # query rows per core
TEMPER = 32.0          # sqrt(d_model)
G = 4                  # head groups of 4
LN_EPS = 1e-3
VSCALE = 8.0           # host scales w_vs by this, pw by 1/this
EXP_S1 = float(8.0 / (TEMPER * np.log(2.0)))
EXP_S2 = 56.0 - 0.37   # rne magic (calibrated on hw)
MAGIC_RCP = 0x7EF30000   # reciprocal seed; 1 Newton -> 0.26% max err
MAGIC_RSQ = 0x5F3759DF   # rsqrt seed; 2 Newtons -> 5e-6

_CACHE = {}


def build(iters=1):
    nc = bacc.Bacc(None, target_bir_lowering=False)
    qt8_d = nc.dram_tensor("qt8", [128, G * 2 * L], F8, kind="ExternalInput")
    wq8_d = nc.dram_tensor("wq8", [128, G * 2 * 1024], F8, kind="ExternalInput")
    wk8_d = nc.dram_tensor("wk8", [128, G * 2 * 1024], F8, kind="ExternalInput")
    wv8_d = nc.dram_tensor("wv8", [128, G * 2 * 1040], F8, kind="ExternalInput")
    pw8_d = nc.dram_tensor("pw8", [65, 8 * 2 * 1024], F8, kind="ExternalInput")
    qres_d = nc.dram_tensor("qres", [HALF, D], F32, kind="ExternalInput")
    out_d = nc.dram_tensor("out", [HALF, D], F32, kind="ExternalOutput")

    # build-time greedy ACT/DVE balancing (ns estimates incl. seq overhead)
    eng_ns = {"A": 0.0, "D": 0.0}

    def pick_ad(rows):
        ca = rows * 0.853 + 124.0
        cd = rows * 1.065 + 108.0
        if eng_ns["A"] + ca <= eng_ns["D"] + cd:
            eng_ns["A"] += ca
            return "A"
        eng_ns["D"] += cd
        return "D"

    def charge_d(rows):
        eng_ns["D"] += rows * 1.065 + 108.0

    def ad_copy(e, dst, src):
        if e == "A":
            nc.scalar.activation(dst, src, AF.Copy)
        else:
            nc.vector.tensor_copy(dst, src)

    with tile.TileContext(nc) as tc:
        with (
            tc.tile_pool(name="p1", bufs=1) as p1,
            tc.tile_pool(name="p2", bufs=2) as p2,
            tc.tile_pool(name="p3", bufs=6) as p3,
            tc.tile_pool(name="p4", bufs=4) as p4,
            tc.tile_pool(name="psS", bufs=2, space="PSUM") as psS,
            tc.tile_pool(name="psO", bufs=2, space="PSUM") as psO,
            tc.tile_pool(name="psA", bufs=2, space="PSUM") as psA,
        ):
            # ---- weight loads ----
            wk8_t = p1.tile([128, G, 2, 1024], F8, name="wk8_t")
            nc.sync.dma_start(wk8_t[:], wk8_d[:])
            qt8_t = p1.tile([128, G, 2, L], F8, name="qt8_t")
            for j in range(G):
                nc.scalar.dma_start(qt8_t[:, j, :, :],
                                    qt8_d[:, j * 2 * L:(j + 1) * 2 * L])
            wq8_t = p1.tile([128, G, 2, 1024], F8, name="wq8_t")
            nc.sync.dma_start(wq8_t[:], wq8_d[:])
            wv8_t = p1.tile([128, G, 2, 1040], F8, name="wv8_t")
            nc.sync.dma_start(wv8_t[:], wv8_d[:])
            pw8_t = p1.tile([65, 8, 2, 1024], F8, name="pw8_t")
            nc.sync.dma_start(pw8_t[:], pw8_d[:])

            for it in range(iters):
                sfx = f"i{it}"
                # per-group fp8 activation stores
                q8 = p1.tile([128, G, 2, HALF], F8, name=f"q8_{sfx}")
                k8 = p1.tile([128, G, 2, L], F8, name=f"k8_{sfx}")
                v8 = p1.tile([128, G, 8, 2, 320], F8, name=f"v8_{sfx}")
                ct8 = p1.tile([65, 8, 2, HALF], F8, name=f"ct8_{sfx}")
                # ones columns of v8 (col 0 of each head's 80-block);
                # the V convert copies only fill cols 1..64.
                for hh in range(4):
                    nc.gpsimd.memset(v8[:, :, :, :, hh * 80], 1.0)

                # layernorm stat tiles
                sums = p1.tile([128, 16], F32, name=f"sums_{sfx}")
                ssq16 = p1.tile([128, 16], F32, name=f"ssq16_{sfx}")
                mu8 = p1.tile([128, 8], F32, name=f"mu8_{sfx}")
                m28 = p1.tile([128, 8], F32, name=f"m28_{sfx}")
                ssq8 = p1.tile([128, 8], F32, name=f"ssq8_{sfx}")
                cs8 = p1.tile([128, 8], F32, name=f"cs8_{sfx}")
                var8 = p1.tile([128, 8], F32, name=f"var8_{sfx}")
                si8 = p1.tile([128, 8], I32, name=f"si8_{sfx}")
                a8 = p1.tile([128, 8], F32, name=f"a8_{sfx}")
                b8 = p1.tile([128, 8], F32, name=f"b8_{sfx}")
                rs8 = p1.tile([128, 8], F32, name=f"rs8_{sfx}")
                rr8 = p1.tile([128, 8], F32, name=f"rr8_{sfx}")
                rec8 = p1.tile([128, 8], F32, name=f"rec8_{sfx}")
                nmr8 = p1.tile([128, 8], F32, name=f"nmr8_{sfx}")
                y_ts = [p1.tile([128, D], F32, name=f"y_{sfx}_{qt}")
                        for qt in range(8)]

                def defer_weave(groups, lag=2):
                    """groups: list of (pe_closure, post_closure|None).
                    Weave so each post lands `lag` slots after its pe part."""
                    items = []
                    pend = []
                    for pe_f, post_f in groups:
                        items.append(pe_f)
                        pend.append(post_f)
                        if len(pend) > lag:
                            f = pend.pop(0)
                            if f is not None:
                                items.append(f)
                    for f in pend:
                        if f is not None:
                            items.append(f)
                    return items

                def kq_groups(g):
                    """K/Q projections for group g: (matmuls, convert)."""
                    groups = []

                    def kq(wt, dst, g, s, blk, nm):
                        hold = {}

                        def mms(hold=hold, g=g, s=s, blk=blk, nm=nm, wt=wt):
                            hold["t"] = psA.tile(
                                [128, 512], F32,
                                name=f"{nm}_{sfx}_{g}_{s}_{blk}", tag="acc")
                            for j in range(G):
                                nc.tensor.matmul(
                                    hold["t"][:],
                                    wt[:, j, :, g * 256 + s * 128:
                                       g * 256 + s * 128 + 128],
                                    qt8_t[:, j, :, blk * 512:(blk + 1) * 512],
                                    start=(j == 0), stop=(j == G - 1),
                                    perf_mode=PM.DoubleRow,
                                )

                        def cv(hold=hold, dst=dst):
                            ad_copy(pick_ad(512), dst, hold["t"][:])

                        return (mms, cv)

                    for s in range(2):
                        for mb in range(4):
                            groups.append(kq(
                                wk8_t, k8[:, g, s, mb * 512:(mb + 1) * 512],
                                g, s, mb, "kp"))
                    for s in range(2):
                        for qb in range(2):
                            groups.append(kq(
                                wq8_t, q8[:, g, s, qb * 512:(qb + 1) * 512],
                                g, s, qb, "qp"))
                    return defer_weave(groups)

                def v_groups(g):
                    """V projection for group g in m-tile order."""
                    groups = []
                    for mt in range(16):
                        hold = {}

                        def vmms(hold=hold, g=g, mt=mt):
                            hold["t"] = psA.tile(
                                [128, 512], F32,
                                name=f"vp_{sfx}_{g}_{mt}", tag="acc")
                            for j in range(G):
                                nc.tensor.matmul(
                                    hold["t"][:, 0:260],
                                    qt8_t[:, j, :, mt * 128:(mt + 1) * 128],
                                    wv8_t[:, j, :, g * 260:(g + 1) * 260],
                                    start=(j == 0), stop=(j == G - 1),
                                    perf_mode=PM.DoubleRow,
                                )

                        def vcv(hold=hold, g=g, mt=mt):
                            ad_copy(
                                pick_ad(256),
                                v8[:, g, mt // 2, mt % 2, :]
                                .rearrange("p (h f) -> p h f", h=4)[:, :, 1:65],
                                hold["t"][:, 0:260]
                                .rearrange("p (h f) -> p h f", h=4)[:, :, 1:65],
                            )

                        groups.append((vmms, vcv))
                    return defer_weave(groups, lag=0)

                def outproj_groups(qts):
                    """Output projection + y-add + squares as (pe, post)."""
                    groups = []
                    for qt in qts:
                        qr_hold = {}

                        def mk_mms(qt, oc, hold, qh):
                            def mms():
                                if oc == 0:
                                    qh["t"] = p4.tile([128, D], F32,
                                                      name=f"qr_{sfx}_{qt}",
                                                      tag="qr")
                                    nc.sync.dma_start(
                                        qh["t"][:],
                                        qres_d[qt * 128:(qt + 1) * 128, :])
                                hold["t"] = psA.tile(
                                    [128, 512], F32,
                                    name=f"yp_{sfx}_{qt}_{oc}", tag="acc")
                                jorder = list(range(8))
                                for n, j in enumerate(jorder):
                                    nc.tensor.matmul(
                                        hold["t"][:],
                                        ct8[:, j, :, qt * 128:(qt + 1) * 128],
                                        pw8_t[:, j, :, oc * 512:(oc + 1) * 512],
                                        start=(n == 0), stop=(n == 7),
                                        perf_mode=PM.DoubleRow,
                                    )
                            return mms

                        def mk_post(qt, oc, hold, qh):
                            def post():
                                nc.vector.scalar_tensor_tensor(
                                    y_ts[qt][:, oc * 512:(oc + 1) * 512],
                                    hold["t"][:], 1.0,
                                    qh["t"][:, oc * 512:(oc + 1) * 512],
                                    ALU.mult, ALU.add,
                                    accum_out=sums[:, 2 * qt + oc:
                                                   2 * qt + oc + 1])
                                charge_d(512)
                                sqt = p2.tile([128, 512], F32,
                                              name=f"sqt_{sfx}_{qt}_{oc}",
                                              tag="sqt")
                                yv = y_ts[qt][:, oc * 512:(oc + 1) * 512]
                                if pick_ad(512) == "A":
                                    nc.scalar.activation(
                                        sqt[:], yv, AF.Square,
                                        accum_out=ssq16[:, 2 * qt + oc:
                                                        2 * qt + oc + 1])
                                else:
                                    nc.vector.scalar_tensor_tensor(
                                        sqt[:], yv, 1.0, yv,
                                        ALU.mult, ALU.mult,
                                        accum_out=ssq16[:, 2 * qt + oc:
                                                        2 * qt + oc + 1])
                            return post

                        for oc in range(2):
                            hold = {}
                            groups.append((mk_mms(qt, oc, hold, qr_hold),
                                           mk_post(qt, oc, hold, qr_hold)))
                    return defer_weave(groups)

                def ln_chain(lo, hi):
                    """sigma chain + finals for q tiles [lo, hi)."""
                    cl = slice(lo, hi)
                    nc.gpsimd.tensor_tensor(ssq8[:, cl],
                                            ssq16[:, 2 * lo:2 * hi:2],
                                            ssq16[:, 2 * lo + 1:2 * hi:2],
                                            ALU.add)
                    nc.gpsimd.tensor_tensor(mu8[:, cl],
                                            sums[:, 2 * lo:2 * hi:2],
                                            sums[:, 2 * lo + 1:2 * hi:2],
                                            ALU.add)
                    nc.gpsimd.tensor_scalar(mu8[:, cl], mu8[:, cl], 1.0 / D,
                                            None, ALU.mult)
                    nc.gpsimd.tensor_tensor(m28[:, cl], mu8[:, cl], mu8[:, cl],
                                            ALU.mult)
                    nc.vector.scalar_tensor_tensor(cs8[:, cl], m28[:, cl],
                                                   -float(D), ssq8[:, cl],
                                                   ALU.mult, ALU.add)
                    nc.gpsimd.tensor_scalar(
                        var8[:, cl], cs8[:, cl],
                        1.0 / ((D - 1) * VSCALE * VSCALE), None, ALU.mult)
                    nc.vector.tensor_scalar(si8[:, cl],
                                            var8[:, cl].bitcast(I32), 1, None,
                                            ALU.arith_shift_right)
                    nc.vector.tensor_scalar(si8[:, cl], si8[:, cl], -1,
                                            MAGIC_RSQ, ALU.mult, ALU.add)
                    r_ap = si8[:, cl].bitcast(F32)
                    nc.gpsimd.tensor_tensor(a8[:, cl], r_ap, r_ap, ALU.mult)
                    nc.gpsimd.tensor_tensor(a8[:, cl], var8[:, cl], a8[:, cl],
                                            ALU.mult)
                    nc.gpsimd.tensor_scalar(a8[:, cl], a8[:, cl], -0.5, 1.5,
                                            ALU.mult, ALU.add)
                    nc.gpsimd.tensor_tensor(b8[:, cl], r_ap, a8[:, cl],
                                            ALU.mult)
                    nc.gpsimd.tensor_tensor(a8[:, cl], b8[:, cl], b8[:, cl],
                                            ALU.mult)
                    nc.gpsimd.tensor_tensor(a8[:, cl], var8[:, cl], a8[:, cl],
                                            ALU.mult)
                    nc.gpsimd.tensor_scalar(a8[:, cl], a8[:, cl], -0.5, 1.5,
                                            ALU.mult, ALU.add)
                    nc.gpsimd.tensor_tensor(rs8[:, cl], b8[:, cl], a8[:, cl],
                                            ALU.mult)
                    nc.gpsimd.tensor_tensor(rr8[:, cl], rs8[:, cl], rs8[:, cl],
                                            ALU.mult)
                    nc.vector.scalar_tensor_tensor(rec8[:, cl], rr8[:, cl],
                                                   -LN_EPS, rs8[:, cl],
                                                   ALU.mult, ALU.add)
                    nc.gpsimd.tensor_tensor(nmr8[:, cl], mu8[:, cl],
                                            rec8[:, cl], ALU.mult)
                    nc.gpsimd.tensor_scalar(nmr8[:, cl], nmr8[:, cl], -1.0,
                                            None, ALU.mult)
                    for qt in range(lo, hi):
                        o_t = p2.tile([128, D], F32, name=f"o_{sfx}_{qt}",
                                      tag="o")
                        if pick_ad(1024) == "A":
                            nc.scalar.activation(
                                o_t[:], y_ts[qt][:], AF.Identity,
                                bias=nmr8[:, qt:qt + 1],
                                scale=rec8[:, qt:qt + 1])
                        else:
                            nc.vector.tensor_scalar(
                                o_t[:], y_ts[qt][:], mu8[:, qt:qt + 1],
                                rec8[:, qt:qt + 1], ALU.subtract, ALU.mult)
                        dq = nc.sync if qt % 2 == 0 else nc.scalar
                        dq.dma_start(out_d[qt * 128:(qt + 1) * 128, :],
                                     o_t[:])

                def emit_head(g, qc, hh, filler, pace, pend):
                    qs = slice(qc * 512, (qc + 1) * 512)
                    p0 = 32 * hh
                    prow = slice(p0, p0 + 32)
                    ot = psO.tile([128, 512], F32,
                                  name=f"ot_{sfx}_{g}_{qc}_{hh}", tag="ot")
                    e8s = {}

                    def attnv(mip):
                        nc.tensor.matmul(
                            ot[0:65, :],
                            v8[:, g, mip, :, hh * 80:hh * 80 + 65],
                            e8s[mip][:],
                            start=(mip == 0), stop=(mip == 7),
                            perf_mode=PM.DoubleRow,
                        )

                    for mip in range(8):
                        sp = psS.tile([128, 1024], F32,
                                      name=f"sp_{sfx}_{g}_{qc}_{hh}_{mip}",
                                      tag="sc")
                        for k in range(2):
                            mi = 2 * mip + k
                            nc.tensor.matmul(
                                sp[:, k * 512:(k + 1) * 512],
                                k8[prow, g, :, mi * 128:(mi + 1) * 128],
                                q8[prow, g, :, qs],
                                start=True, stop=True,
                                perf_mode=PM.DoubleRow,
                                tile_position=(p0, 0),
                            )
                        e8 = p3.tile([128, 2, 512], F8,
                                     name=f"e8_{sfx}_{g}_{qc}_{hh}_{mip}",
                                     tag="e8")
                        e8s[mip] = e8
                        if pick_ad(1024) == "A":
                            nc.scalar.activation(
                                e8[:].rearrange("p s f -> p (s f)"),
                                sp[:], AF.Exp, scale=1.0 / TEMPER)
                        else:
                            nc.vector.tensor_scalar(
                                e8[:].bitcast(I8).rearrange("p s f -> p (s f)"),
                                sp[:], EXP_S1, EXP_S2, ALU.mult, ALU.add)
                        if mip == 1 and pend is not None:
                            pend()
                            pend = None
                        for _ in range(pace):
                            try:
                                next(filler)()
                            except StopIteration:
                                break
                        if mip >= 1:
                            attnv(mip - 1)
                    attnv(7)
                    if pend is not None:
                        pend()

                    def tail(g=g, qc=qc, hh=hh, ot=ot):
                        # drain -> recip(denominator) -> bcast -> scale
                        stage = p2.tile([65, 512], F32,
                                        name=f"st_{sfx}_{g}_{qc}_{hh}",
                                        tag="otst")
                        ad_copy(pick_ad(512), stage[:], ot[0:65, :])
                        rci = p2.tile([1, 512], I32,
                                      name=f"rci_{sfx}_{g}_{qc}_{hh}",
                                      tag="rci")
                        nc.gpsimd.tensor_scalar(rci[:],
                                                stage[0:1, :].bitcast(I32),
                                                -1, MAGIC_RCP,
                                                ALU.mult, ALU.add)
                        tt = p2.tile([1, 512], F32,
                                     name=f"tt_{sfx}_{g}_{qc}_{hh}", tag="tt")
                        nc.gpsimd.tensor_tensor(tt[:], stage[0:1, :],
                                                rci[:].bitcast(F32), ALU.mult)
                        nc.gpsimd.tensor_scalar(tt[:], tt[:], -1.0, 2.0,
                                                ALU.mult, ALU.add)
                        rc = p2.tile([1, 512], F32,
                                     name=f"rc_{sfx}_{g}_{qc}_{hh}", tag="rc")
                        nc.gpsimd.tensor_tensor(rc[:], rci[:].bitcast(F32),
                                                tt[:], ALU.mult)
                        rcb = p2.tile([65, 512], F32,
                                      name=f"rcb_{sfx}_{g}_{qc}_{hh}",
                                      tag="rcb")
                        nc.gpsimd.partition_broadcast(rcb[:], rc[:])
                        h = 4 * g + hh
                        nc.gpsimd.tensor_tensor(
                            ct8[:, h // 2, h % 2, qs], stage[:], rcb[:],
                            ALU.mult)

                    return tail

                # ---- emission: K/Q(0) upfront; attention filler = V(g)
                # then K/Q(g+1) projections / output proj ----
                for f in kq_groups(0):
                    f()
                pend = None
                for g in range(G):
                    fill_list = v_groups(g)
                    if g < G - 1:
                        fill_list = fill_list + kq_groups(g + 1)
                    filler = iter(fill_list)
                    pace = 1
                    for qc in range(2):
                        if g == G - 1 and qc == 1:
                            if pend is not None:
                                pend()
                                pend = None
                            for f in filler:
                                f()
                            fill_list = outproj_groups(range(4))
                            filler = iter(fill_list)
                        for hh in range(4):
                            hp = 4 if (qc == 0 and hh == 0) else pace
                            pend = emit_head(g, qc, hh, filler, hp, pend)
                    for f in filler:
                        f()
                if pend is not None:
                    pend()

                # ---- tail: finals for qt 0-3 overlap outproj qt 4-7 ----
                ln_chain(0, 4)
                for f in outproj_groups(range(4, 8)):
                    f()
                ln_chain(4, 8)

    nc.compile()
    return nc


def _get_nc():
    if "nc" not in _CACHE:
        _CACHE["nc"] = build()
    return _CACHE["nc"]


def _prep_shared(w_qs, w_ks, w_vs, proj_w):
    """fp8 weight layouts: rows d -> [p, j, s] with d = 256j + 128s + p."""
    def dsplit(a):  # [1024, N] -> [128, 4*2*N]
        n = a.shape[1]
        return np.ascontiguousarray(
            a.reshape(G, 2, 128, n).transpose(2, 0, 1, 3).reshape(128, -1)
        )

    # wq/wk cols: g*256 + (dk//32)*128 + hh*32 + dk%32  <- head 4g+hh
    wq = np.empty((D, H * DK), dtype=np.float32)
    wk = np.empty((D, H * DK), dtype=np.float32)
    for g in range(G):
        for s in range(2):
            for hh in range(4):
                c0 = g * 256 + s * 128 + hh * 32
                wq[:, c0:c0 + 32] = w_qs[4 * g + hh, :, 32 * s:32 * s + 32]
                wk[:, c0:c0 + 32] = w_ks[4 * g + hh, :, 32 * s:32 * s + 32]
    # wv cols: g*260 + hh*65 + (1+dv); col hh*65 is the ones slot
    wv = np.zeros((D, G * 4 * 65), dtype=np.float32)
    for g in range(G):
        for hh in range(4):
            c0 = g * 260 + hh * 65
            wv[:, c0 + 1:c0 + 65] = w_vs[4 * g + hh] * VSCALE
    # pw8 [65, 8, 2, 1024]: row p=0 zero (denominator slot), p=1+dv maps
    # to concat row (2j+s)*64+dv of proj_w.T
    pwT = proj_w.T.astype(np.float32)  # [c, o]
    pw8 = np.zeros((65, 8, 2, D), dtype=np.float32)
    for j in range(8):
        for s in range(2):
            h = 2 * j + s
            pw8[1:65, j, s, :] = pwT[h * 64:(h + 1) * 64, :]
    pw8 = pw8.reshape(65, -1)
    wq8 = dsplit(wq).astype(E4M3)
    wk8 = dsplit(wk).astype(E4M3)
    wv8 = dsplit(wv).astype(E4M3)
    pw8 = np.ascontiguousarray(pw8).astype(E4M3)
    return wq8, wk8, wv8, pw8


def kernel(q, w_qs, w_ks, w_vs, proj_w, proj_b, ln_a, ln_b, **kw):
    q = np.asarray(q, dtype=np.float32)
    w_qs = np.asarray(w_qs, dtype=np.float32)
    w_ks = np.asarray(w_ks, dtype=np.float32)
    w_vs = np.asarray(w_vs, dtype=np.float32)
    proj_w = np.asarray(proj_w, dtype=np.float32)
    proj_b = np.asarray(proj_b, dtype=np.float32)
    ln_a = np.asarray(ln_a, dtype=np.float32)
    ln_b = np.asarray(ln_b, dtype=np.float32)

    wq8, wk8, wv8, pw8 = _prep_shared(w_qs, w_ks, w_vs, proj_w)

    in_maps = []
    for c in range(8):
        b, half = c // 2, c % 2
        qbT = q[b].T  # [D, L]
        qcat = np.concatenate(
            [qbT[:, half * HALF:(half + 1) * HALF],
             qbT[:, (1 - half) * HALF:(2 - half) * HALF]], axis=1)
        qt8 = np.ascontiguousarray(
            qcat.reshape(G, 2, 128, L).transpose(2, 0, 1, 3).reshape(128, -1)
        ).astype(E4M3)
        qres_c = np.ascontiguousarray(
            (q[b, half * HALF:(half + 1) * HALF, :] + proj_b[None, :]) * VSCALE)
        in_maps.append({
            "qt8": qt8, "qres": qres_c,
            "wq8": wq8, "wk8": wk8, "wv8": wv8, "pw8": pw8,
        })

    nc = _get_nc()
    res = run_bass_kernel_spmd(nc, in_maps, core_ids=list(range(8))).results

    out = np.empty((B, L, D), dtype=np.float32)
    for c in range(8):
        b, half = c // 2, c % 2
        out[b, half * HALF:(half + 1) * HALF, :] = res[c]["out"]
    # ln affine on host
    out = out * (ln_a[None, None, :] / VSCALE) + ln_b[None, None, :]
    return out


# revision 31
# speedup vs baseline: 1.0198x; 1.0001x over previous
"""MultiHeadAttention TRN2 Bass kernel (8 NeuronCores), fp8 DoubleRow edition.

Sharding: core c = (batch b = c//2, query-half = c%2). Each core computes
K/V for its full batch (2048 keys) and attention + output projection + LN
for its 1024 query rows. No collectives; host gathers per-core outputs.

All heavy matmuls run in fp8e4m3 with MatmulPerfMode.DoubleRow (0.5 PE
cycles per output row, 256-wide contraction per instruction):
  - Q/K projections:  out[4head*32dk half, q|m] over D=1024 (4 DR chunks)
  - V projection:     Vaug[m, 4head*65] (col 0 of each head = ones so the
                      softmax denominator lands at partition 0 of OT)
  - scores:           S[m, q] per head, dk=64 = 2x32 DR slices at
                      tile_position (32*hh, 0)
  - attn*V:           OT[65, q] accumulated over 8 DR m-pair chunks
  - output proj:      Y[q, o] over 8 chunks of [65,2] (denominator row is
                      multiplied by zeroed pw rows)

PSUM can only be read by ACT and DVE, so exp(S/32), the K/Q/V fp8
converts, OT drains, and residual adds are greedily load-balanced
between those two engines at build time; Pool (gpsimd) handles all
SBUF-side work (denominator broadcast + CT scale, LN stats + finals).
exp on DVE uses the int8 bit trick:
  i8 = rne(S*8/(32*ln2) + 55.63), bitcast int8 -> fp8e4m3 ~= exp(S/32)
(the denominator sums the same approximated values -> consistent).

LayerNorm: device computes z = (y - mu)/(sigma_ddof1 + eps); the ln_a/ln_b
affine is applied on host. V is scaled x8 on host (pw /8) to keep CT
inside the fp8 normal range.
"""
import numpy as np
import ml_dtypes

import concourse.bass as bass
import concourse.mybir as mybir
import concourse.tile as tile
from concourse import bacc
from concourse.bass_utils import run_bass_kernel_spmd

F32 = mybir.dt.float32
F32R = mybir.dt.float32r
F8 = mybir.dt.float8e4
I8 = mybir.dt.int8
I32 = mybir.dt.int32
AF = mybir.ActivationFunctionType
ALU = mybir.AluOpType
PM = mybir.MatmulPerfMode
E4M3 = ml_dtypes.float8_e4m3

B, L, D = 4, 2048, 1024
H, DK = 16, 64
HALF = 1024            # query rows per core
TEMPER = 32.0          # sqrt(d_model)
G = 4                  # head groups of 4
LN_EPS = 1e-3
VSCALE = 8.0           # host scales w_vs by this, pw by 1/this
EXP_S1 = float(8.0 / (TEMPER * np.log(2.0)))
EXP_S2 = 56.0 - 0.37   # rne magic (calibrated on hw)
MAGIC_RCP = 0x7EF30000   # reciprocal seed; 1 Newton -> 0.26% max err
MAGIC_RSQ = 0x5F3759DF   # rsqrt seed; 2 Newtons -> 5e-6

_CACHE = {}


def build(iters=1):
    nc = bacc.Bacc(None, target_bir_lowering=False)
    qt8_d = nc.dram_tensor("qt8", [128, G * 2 * L], F8, kind="ExternalInput")
    wq8_d = nc.dram_tensor("wq8", [128, G * 2 * 1024], F8, kind="ExternalInput")
    wk8_d = nc.dram_tensor("wk8", [128, G * 2 * 1024], F8, kind="ExternalInput")
    wv8_d = nc.dram_tensor("wv8", [128, G * 2 * 1040], F8, kind="ExternalInput")
    pw8_d = nc.dram_tensor("pw8", [65, 8 * 2 * 1024], F8, kind="ExternalInput")
    qres_d = nc.dram_tensor("qres", [HALF, D], F32, kind="ExternalInput")
    out_d = nc.dram_tensor("out", [HALF, D], F32, kind="ExternalOutput")

    # build-time greedy ACT/DVE balancing (ns estimates incl. seq overhead)
    eng_ns = {"A": 0.0, "D": 0.0}

    def pick_ad(rows):
        ca = rows * 0.853 + 124.0
        cd = rows * 1.065 + 108.0
        if eng_ns["A"] + ca <= eng_ns["D"] + cd:
            eng_ns["A"] += ca
            return "A"
        eng_ns["D"] += cd
        return "D"

    def charge_d(rows):
        eng_ns["D"] += rows * 1.065 + 108.0

    def ad_copy(e, dst, src):
        if e == "A":
            nc.scalar.activation(dst, src, AF.Copy)
        else:
            nc.vector.tensor_copy(dst, src)

    with tile.TileContext(nc) as tc:
        with (
            tc.tile_pool(name="p1", bufs=1) as p1,
            tc.tile_pool(name="p2", bufs=2) as p2,
            tc.tile_pool(name="p3", bufs=6) as p3,
            tc.tile_pool(name="p4", bufs=4) as p4,
            tc.tile_pool(name="psS", bufs=2, space="PSUM") as psS,
            tc.tile_pool(name="psO", bufs=2, space="PSUM") as psO,
            tc.tile_pool(name="psA", bufs=2, space="PSUM") as psA,
        ):
            # ---- weight loads ----
            wk8_t = p1.tile([128, G, 2, 1024], F8, name="wk8_t")
            nc.scalar.dma_start(wk8_t[:], wk8_d[:])
            qt8_t = p1.tile([128, G, 2, L], F8, name="qt8_t")
            for j in range(G):
                dq = nc.scalar if j == 0 else nc.sync
                dq.dma_start(qt8_t[:, j, :, :],
                             qt8_d[:, j * 2 * L:(j + 1) * 2 * L])
            wq8_t = p1.tile([128, G, 2, 1024], F8, name="wq8_t")
            nc.sync.dma_start(wq8_t[:], wq8_d[:])
            wv8_t = p1.tile([128, G, 2, 1040], F8, name="wv8_t")
            nc.sync.dma_start(wv8_t[:], wv8_d[:])
            pw8_t = p1.tile([65, 8, 2, 1024], F8, name="pw8_t")
            nc.sync.dma_start(pw8_t[:], pw8_d[:])

            for it in range(iters):
                sfx = f"i{it}"
                # per-group fp8 activation stores
                q8 = p1.tile([128, G, 2, HALF], F8, name=f"q8_{sfx}")
                k8 = p1.tile([128, G, 2, L], F8, name=f"k8_{sfx}")
                v8 = p1.tile([128, G, 8, 2, 320], F8, name=f"v8_{sfx}")
                ct8 = p1.tile([65, 8, 2, HALF], F8, name=f"ct8_{sfx}")
                # ones columns of v8 (col 0 of each head's 80-block);
                # the V convert copies only fill cols 1..64.
                for hh in range(4):
                    nc.gpsimd.memset(v8[:, :, :, :, hh * 80], 1.0)

                # layernorm stat tiles
                sums = p1.tile([128, 16], F32, name=f"sums_{sfx}")
                ssq16 = p1.tile([128, 16], F32, name=f"ssq16_{sfx}")
                mu8 = p1.tile([128, 8], F32, name=f"mu8_{sfx}")
                m28 = p1.tile([128, 8], F32, name=f"m28_{sfx}")
                ssq8 = p1.tile([128, 8], F32, name=f"ssq8_{sfx}")
                cs8 = p1.tile([128, 8], F32, name=f"cs8_{sfx}")
                var8 = p1.tile([128, 8], F32, name=f"var8_{sfx}")
                si8 = p1.tile([128, 8], I32, name=f"si8_{sfx}")
                a8 = p1.tile([128, 8], F32, name=f"a8_{sfx}")
                b8 = p1.tile([128, 8], F32, name=f"b8_{sfx}")
                rs8 = p1.tile([128, 8], F32, name=f"rs8_{sfx}")
                rr8 = p1.tile([128, 8], F32, name=f"rr8_{sfx}")
                rec8 = p1.tile([128, 8], F32, name=f"rec8_{sfx}")
                nmr8 = p1.tile([128, 8], F32, name=f"nmr8_{sfx}")
                y_ts = [p1.tile([128, D], F32, name=f"y_{sfx}_{qt}")
                        for qt in range(8)]

                def defer_weave(groups, lag=2):
                    """groups: list of (pe_closure, post_closure|None).
                    Weave so each post lands `lag` slots after its pe part."""
                    items = []
                    pend = []
                    for pe_f, post_f in groups:
                        items.append(pe_f)
                        pend.append(post_f)
                        if len(pend) > lag:
                            f = pend.pop(0)
                            if f is not None:
                                items.append(f)
                    for f in pend:
                        if f is not None:
                            items.append(f)
                    return items

                def kq_groups(g):
                    """K/Q projections for group g: (matmuls, convert)."""
                    groups = []

                    def kq(wt, dst, g, s, blk, nm):
                        hold = {}

                        def mms(hold=hold, g=g, s=s, blk=blk, nm=nm, wt=wt):
                            hold["t"] = psA.tile(
                                [128, 512], F32,
                                name=f"{nm}_{sfx}_{g}_{s}_{blk}", tag="acc")
                            for j in range(G):
                                nc.tensor.matmul(
                                    hold["t"][:],
                                    wt[:, j, :, g * 256 + s * 128:
                                       g * 256 + s * 128 + 128],
                                    qt8_t[:, j, :, blk * 512:(blk + 1) * 512],
                                    start=(j == 0), stop=(j == G - 1),
                                    perf_mode=PM.DoubleRow,
                                )

                        def cv(hold=hold, dst=dst):
                            ad_copy(pick_ad(512), dst, hold["t"][:])

                        return (mms, cv)

                    for s in range(2):
                        for mb in range(4):
                            groups.append(kq(
                                wk8_t, k8[:, g, s, mb * 512:(mb + 1) * 512],
                                g, s, mb, "kp"))
                    for s in range(2):
                        for qb in range(2):
                            groups.append(kq(
                                wq8_t, q8[:, g, s, qb * 512:(qb + 1) * 512],
                                g, s, qb, "qp"))
                    return defer_weave(groups)

                def v_groups(g):
                    """V projection for group g in m-tile order."""
                    groups = []
                    for mt in range(16):
                        hold = {}

                        def vmms(hold=hold, g=g, mt=mt):
                            hold["t"] = psA.tile(
                                [128, 512], F32,
                                name=f"vp_{sfx}_{g}_{mt}", tag="acc")
                            for j in range(G):
                                nc.tensor.matmul(
                                    hold["t"][:, 0:260],
                                    qt8_t[:, j, :, mt * 128:(mt + 1) * 128],
                                    wv8_t[:, j, :, g * 260:(g + 1) * 260],
                                    start=(j == 0), stop=(j == G - 1),
                                    perf_mode=PM.DoubleRow,
                                )

                        def vcv(hold=hold, g=g, mt=mt):
                            ad_copy(
                                pick_ad(256),
                                v8[:, g, mt // 2, mt % 2, :]
                                .rearrange("p (h f) -> p h f", h=4)[:, :, 1:65],
                                hold["t"][:, 0:260]
                                .rearrange("p (h f) -> p h f", h=4)[:, :, 1:65],
                            )

                        groups.append((vmms, vcv))
                    return defer_weave(groups, lag=0)

                def outproj_groups(qts):
                    """Output projection + y-add + squares as (pe, post)."""
                    groups = []
                    for qt in qts:
                        qr_hold = {}

                        def mk_mms(qt, oc, hold, qh):
                            def mms():
                                if oc == 0:
                                    qh["t"] = p4.tile([128, D], F32,
                                                      name=f"qr_{sfx}_{qt}",
                                                      tag="qr")
                                    nc.sync.dma_start(
                                        qh["t"][:],
                                        qres_d[qt * 128:(qt + 1) * 128, :])
                                hold["t"] = psA.tile(
                                    [128, 512], F32,
                                    name=f"yp_{sfx}_{qt}_{oc}", tag="acc")
                                jorder = list(range(8))
                                for n, j in enumerate(jorder):
                                    nc.tensor.matmul(
                                        hold["t"][:],
                                        ct8[:, j, :, qt * 128:(qt + 1) * 128],
                                        pw8_t[:, j, :, oc * 512:(oc + 1) * 512],
                                        start=(n == 0), stop=(n == 7),
                                        perf_mode=PM.DoubleRow,
                                    )
                            return mms

                        def mk_post(qt, oc, hold, qh):
                            def post():
                                nc.vector.scalar_tensor_tensor(
                                    y_ts[qt][:, oc * 512:(oc + 1) * 512],
                                    hold["t"][:], 1.0,
                                    qh["t"][:, oc * 512:(oc + 1) * 512],
                                    ALU.mult, ALU.add,
                                    accum_out=sums[:, 2 * qt + oc:
                                                   2 * qt + oc + 1])
                                charge_d(512)
                                sqt = p2.tile([128, 512], F32,
                                              name=f"sqt_{sfx}_{qt}_{oc}",
                                              tag="sqt")
                                yv = y_ts[qt][:, oc * 512:(oc + 1) * 512]
                                if pick_ad(512) == "A":
                                    nc.scalar.activation(
                                        sqt[:], yv, AF.Square,
                                        accum_out=ssq16[:, 2 * qt + oc:
                                                        2 * qt + oc + 1])
                                else:
                                    nc.vector.scalar_tensor_tensor(
                                        sqt[:], yv, 1.0, yv,
                                        ALU.mult, ALU.mult,
                                        accum_out=ssq16[:, 2 * qt + oc:
                                                        2 * qt + oc + 1])
                            return post

                        for oc in range(2):
                            hold = {}
                            groups.append((mk_mms(qt, oc, hold, qr_hold),
                                           mk_post(qt, oc, hold, qr_hold)))
                    return defer_weave(groups)

                def ln_chain(lo, hi):
                    """sigma chain + finals for q tiles [lo, hi)."""
                    cl = slice(lo, hi)
                    nc.gpsimd.tensor_tensor(ssq8[:, cl],
                                            ssq16[:, 2 * lo:2 * hi:2],
                                            ssq16[:, 2 * lo + 1:2 * hi:2],
                                            ALU.add)
                    nc.gpsimd.tensor_tensor(mu8[:, cl],
                                            sums[:, 2 * lo:2 * hi:2],
                                            sums[:, 2 * lo + 1:2 * hi:2],
                                            ALU.add)
                    nc.gpsimd.tensor_scalar(mu8[:, cl], mu8[:, cl], 1.0 / D,
                                            None, ALU.mult)
                    nc.gpsimd.tensor_tensor(m28[:, cl], mu8[:, cl], mu8[:, cl],
                                            ALU.mult)
                    nc.vector.scalar_tensor_tensor(cs8[:, cl], m28[:, cl],
                                                   -float(D), ssq8[:, cl],
                                                   ALU.mult, ALU.add)
                    nc.gpsimd.tensor_scalar(
                        var8[:, cl], cs8[:, cl],
                        1.0 / ((D - 1) * VSCALE * VSCALE), None, ALU.mult)
                    nc.vector.tensor_scalar(si8[:, cl],
                                            var8[:, cl].bitcast(I32), 1, None,
                                            ALU.arith_shift_right)
                    nc.vector.tensor_scalar(si8[:, cl], si8[:, cl], -1,
                                            MAGIC_RSQ, ALU.mult, ALU.add)
                    r_ap = si8[:, cl].bitcast(F32)
                    nc.gpsimd.tensor_tensor(a8[:, cl], r_ap, r_ap, ALU.mult)
                    nc.gpsimd.tensor_tensor(a8[:, cl], var8[:, cl], a8[:, cl],
                                            ALU.mult)
                    nc.gpsimd.tensor_scalar(a8[:, cl], a8[:, cl], -0.5, 1.5,
                                            ALU.mult, ALU.add)
                    nc.gpsimd.tensor_tensor(b8[:, cl], r_ap, a8[:, cl],
                                            ALU.mult)
                    nc.gpsimd.tensor_tensor(a8[:, cl], b8[:, cl], b8[:, cl],
                                            ALU.mult)
                    nc.gpsimd.tensor_tensor(a8[:, cl], var8[:, cl], a8[:, cl],
                                            ALU.mult)
                    nc.gpsimd.tensor_scalar(a8[:, cl], a8[:, cl], -0.5, 1.5,
                                            ALU.mult, ALU.add)
                    nc.gpsimd.tensor_tensor(rs8[:, cl], b8[:, cl], a8[:, cl],
                                            ALU.mult)
                    nc.gpsimd.tensor_tensor(rr8[:, cl], rs8[:, cl], rs8[:, cl],
                                            ALU.mult)
                    nc.vector.scalar_tensor_tensor(rec8[:, cl], rr8[:, cl],
                                                   -LN_EPS, rs8[:, cl],
                                                   ALU.mult, ALU.add)
                    nc.gpsimd.tensor_tensor(nmr8[:, cl], mu8[:, cl],
                                            rec8[:, cl], ALU.mult)
                    nc.gpsimd.tensor_scalar(nmr8[:, cl], nmr8[:, cl], -1.0,
                                            None, ALU.mult)
                    for qt in range(lo, hi):
                        o_t = p2.tile([128, D], F32, name=f"o_{sfx}_{qt}",
                                      tag="o")
                        if pick_ad(1024) == "A":
                            nc.scalar.activation(
                                o_t[:], y_ts[qt][:], AF.Identity,
                                bias=nmr8[:, qt:qt + 1],
                                scale=rec8[:, qt:qt + 1])
                        else:
                            nc.vector.tensor_scalar(
                                o_t[:], y_ts[qt][:], mu8[:, qt:qt + 1],
                                rec8[:, qt:qt + 1], ALU.subtract, ALU.mult)
                        dq = nc.sync if qt % 2 == 0 else nc.scalar
                        dq.dma_start(out_d[qt * 128:(qt + 1) * 128, :],
                                     o_t[:])

                def emit_head(g, qc, hh, filler, pace, pend):
                    qs = slice(qc * 512, (qc + 1) * 512)
                    p0 = 32 * hh
                    prow = slice(p0, p0 + 32)
                    ot = psO.tile([128, 512], F32,
                                  name=f"ot_{sfx}_{g}_{qc}_{hh}", tag="ot")
                    e8s = {}

                    def attnv(mip):
                        nc.tensor.matmul(
                            ot[0:65, :],
                            v8[:, g, mip, :, hh * 80:hh * 80 + 65],
                            e8s[mip][:],
                            start=(mip == 0), stop=(mip == 7),
                            perf_mode=PM.DoubleRow,
                        )

                    for mip in range(8):
                        sp = psS.tile([128, 1024], F32,
                                      name=f"sp_{sfx}_{g}_{qc}_{hh}_{mip}",
                                      tag="sc")
                        for k in range(2):
                            mi = 2 * mip + k
                            nc.tensor.matmul(
                                sp[:, k * 512:(k + 1) * 512],
                                k8[prow, g, :, mi * 128:(mi + 1) * 128],
                                q8[prow, g, :, qs],
                                start=True, stop=True,
                                perf_mode=PM.DoubleRow,
                                tile_position=(p0, 0),
                            )
                        e8 = p3.tile([128, 2, 512], F8,
                                     name=f"e8_{sfx}_{g}_{qc}_{hh}_{mip}",
                                     tag="e8")
                        e8s[mip] = e8
                        if pick_ad(1024) == "A":
                            nc.scalar.activation(
                                e8[:].rearrange("p s f -> p (s f)"),
                                sp[:], AF.Exp, scale=1.0 / TEMPER)
                        else:
                            nc.vector.tensor_scalar(
                                e8[:].bitcast(I8).rearrange("p s f -> p (s f)"),
                                sp[:], EXP_S1, EXP_S2, ALU.mult, ALU.add)
                        if mip == 1 and pend is not None:
                            pend()
                            pend = None
                        for _ in range(pace):
                            try:
                                next(filler)()
                            except StopIteration:
                                break
                        if mip >= 1:
                            attnv(mip - 1)
                    attnv(7)
                    if pend is not None:
                        pend()

                    def tail(g=g, qc=qc, hh=hh, ot=ot):
                        # drain -> recip(denominator) -> bcast -> scale
                        stage = p2.tile([65, 512], F32,
                                        name=f"st_{sfx}_{g}_{qc}_{hh}",
                                        tag="otst")
                        ad_copy(pick_ad(512), stage[:], ot[0:65, :])
                        rci = p2.tile([1, 512], I32,
                                      name=f"rci_{sfx}_{g}_{qc}_{hh}",
                                      tag="rci")
                        nc.gpsimd.tensor_scalar(rci[:],
                                                stage[0:1, :].bitcast(I32),
                                                -1, MAGIC_RCP,
                                                ALU.mult, ALU.add)
                        tt = p2.tile([1, 512], F32,
                                     name=f"tt_{sfx}_{g}_{qc}_{hh}", tag="tt")
                        nc.gpsimd.tensor_tensor(tt[:], stage[0:1, :],
                                                rci[:].bitcast(F32), ALU.mult)
                        nc.gpsimd.tensor_scalar(tt[:], tt[:], -1.0, 2.0,
                                                ALU.mult, ALU.add)
                        rc = p2.tile([1, 512], F32,
                                     name=f"rc_{sfx}_{g}_{qc}_{hh}", tag="rc")
                        nc.gpsimd.tensor_tensor(rc[:], rci[:].bitcast(F32),
                                                tt[:], ALU.mult)
                        rcb = p2.tile([65, 512], F32,
                                      name=f"rcb_{sfx}_{g}_{qc}_{hh}",
                                      tag="rcb")
                        nc.gpsimd.partition_broadcast(rcb[:], rc[:])
                        h = 4 * g + hh
                        nc.gpsimd.tensor_tensor(
                            ct8[:, h // 2, h % 2, qs], stage[:], rcb[:],
                            ALU.mult)

                    return tail

                # ---- emission: K/Q(0) upfront; attention filler = V(g)
                # then K/Q(g+1) projections / output proj ----
                for f in kq_groups(0):
                    f()
                pend = None
                for g in range(G):
                    fill_list = v_groups(g)
                    if g < G - 1:
                        fill_list = fill_list + kq_groups(g + 1)
                    filler = iter(fill_list)
                    pace = 1
                    for qc in range(2):
                        if g == G - 1 and qc == 1:
                            if pend is not None:
                                pend()
                                pend = None
                            for f in filler:
                                f()
                            fill_list = outproj_groups(range(4))
                            filler = iter(fill_list)
                        for hh in range(4):
                            hp = 4 if (qc == 0 and hh == 0) else pace
                            pend = emit_head(g, qc, hh, filler, hp, pend)
                    for f in filler:
                        f()
                if pend is not None:
                    pend()

                # ---- tail: finals for qt 0-3 overlap outproj qt 4-7 ----
                ln_chain(0, 4)
                for f in outproj_groups(range(4, 8)):
                    f()
                ln_chain(4, 8)

    nc.compile()
    return nc


def _get_nc():
    if "nc" not in _CACHE:
        _CACHE["nc"] = build()
    return _CACHE["nc"]


def _prep_shared(w_qs, w_ks, w_vs, proj_w):
    """fp8 weight layouts: rows d -> [p, j, s] with d = 256j + 128s + p."""
    def dsplit(a):  # [1024, N] -> [128, 4*2*N]
        n = a.shape[1]
        return np.ascontiguousarray(
            a.reshape(G, 2, 128, n).transpose(2, 0, 1, 3).reshape(128, -1)
        )

    # wq/wk cols: g*256 + (dk//32)*128 + hh*32 + dk%32  <- head 4g+hh
    wq = np.empty((D, H * DK), dtype=np.float32)
    wk = np.empty((D, H * DK), dtype=np.float32)
    for g in range(G):
        for s in range(2):
            for hh in range(4):
                c0 = g * 256 + s * 128 + hh * 32
                wq[:, c0:c0 + 32] = w_qs[4 * g + hh, :, 32 * s:32 * s + 32]
                wk[:, c0:c0 + 32] = w_ks[4 * g + hh, :, 32 * s:32 * s + 32]
    # wv cols: g*260 + hh*65 + (1+dv); col hh*65 is the ones slot
    wv = np.zeros((D, G * 4 * 65), dtype=np.float32)
    for g in range(G):
        for hh in range(4):
            c0 = g * 260 + hh * 65
            wv[:, c0 + 1:c0 + 65] = w_vs[4 * g + hh] * VSCALE
    # pw8 [65, 8, 2, 1024]: row p=0 zero (denominator slot), p=1+dv maps
    # to concat row (2j+s)*64+dv of proj_w.T
    pwT = proj_w.T.astype(np.float32)  # [c, o]
    pw8 = np.zeros((65, 8, 2, D), dtype=np.float32)
    for j in range(8):
        for s in range(2):
            h = 2 * j + s
            pw8[1:65, j, s, :] = pwT[h * 64:(h + 1) * 64, :]
    pw8 = pw8.reshape(65, -1)
    wq8 = dsplit(wq).astype(E4M3)
    wk8 = dsplit(wk).astype(E4M3)
    wv8 = dsplit(wv).astype(E4M3)
    pw8 = np.ascontiguousarray(pw8).astype(E4M3)
    return wq8, wk8, wv8, pw8


def kernel(q, w_qs, w_ks, w_vs, proj_w, proj_b, ln_a, ln_b, **kw):
    q = np.asarray(q, dtype=np.float32)
    w_qs = np.asarray(w_qs, dtype=np.float32)
    w_ks = np.asarray(w_ks, dtype=np.float32)
    w_vs = np.asarray(w_vs, dtype=np.float32)
    proj_w = np.asarray(proj_w, dtype=np.float32)
    proj_b = np.asarray(proj_b, dtype=np.float32)
    ln_a = np.asarray(ln_a, dtype=np.float32)
    ln_b = np.asarray(ln_b, dtype=np.float32)

    wq8, wk8, wv8, pw8 = _prep_shared(w_qs, w_ks, w_vs, proj_w)

    in_maps = []
    for c in range(8):
        b, half = c // 2, c % 2
        qbT = q[b].T  # [D, L]
        qcat = np.concatenate(
            [qbT[:, half * HALF:(half + 1) * HALF],
             qbT[:, (1 - half) * HALF:(2 - half) * HALF]], axis=1)
        qt8 = np.ascontiguousarray(
            qcat.reshape(G, 2, 128, L).transpose(2, 0, 1, 3).reshape(128, -1)
        ).astype(E4M3)
        qres_c = np.ascontiguousarray(
            (q[b, half * HALF:(half + 1) * HALF, :] + proj_b[None, :]) * VSCALE)
        in_maps.append({
            "qt8": qt8, "qres": qres_c,
            "wq8": wq8, "wk8": wk8, "wv8": wv8, "pw8": pw8,
        })

    nc = _get_nc()
    res = run_bass_kernel_spmd(nc, in_maps, core_ids=list(range(8))).results

    out = np.empty((B, L, D), dtype=np.float32)
    for c in range(8):
        b, half = c // 2, c % 2
        out[b, half * HALF:(half + 1) * HALF, :] = res[c]["out"]
    # ln affine on host
    out = out * (ln_a[None, None, :] / VSCALE) + ln_b[None, None, :]
    return out
